# revision 1
# baseline (speedup 1.0000x reference)
"""Trainium2 Bass kernel for GroupedVectorSA (gnn message passing).

Self-contained: accepts FULL inputs (as produced by setup_inputs()), shards
across 8 NeuronCores internally (batch b = core//4, quarter of N = core%4),
runs one SPMD Bass/Tile program via bass_utils.run_bass_kernel_spmd, and
reassembles the full [B, N, C] output.

Math per core (points n in its quarter, full k/v of its batch):
  q = relu(bn(feats @ wq + bq)); k = relu(bn(feats @ wk + bk)); v = feats @ wv + bv
  gather k/v/coords rows by index; pos = coords[n] - coords[idx]
  pem = relu(bn(pos @ pm_w1 + pm_b1)) @ pm_w2 + pm_b2
  peb0 = relu(bn(pos @ pb_w1 + pb_b1)) @ pb_w2          (pb_b2 folded)
  rel = (kg - q)*pem + peb0
  w = softmax(relu(bn(rel @ we_w1 + we_b1')) @ we_w2 + we_b2, over S)
  out = sum_s w * (vg + peb0) + pb_b2      (softmax weights sum to 1)

BN is folded on the host into per-channel affine scale/bias (eval mode).
"""

import os
import sys

import numpy as np

try:
    import concourse  # noqa: F401
except ImportError:
    sys.path.insert(0, "/opt/trn_rl_repo")

import ml_dtypes

import concourse.bacc as bacc
import concourse.bass as bass  # noqa: F401
import concourse.mybir as mybir
import concourse.tile as tile
from concourse import bass_utils

F32 = mybir.dt.float32
BF16 = mybir.dt.bfloat16
FP16 = mybir.dt.float16
I16 = mybir.dt.int16

NP_BF16 = ml_dtypes.bfloat16

EPS = 1e-5
B, N, S, C, G = 2, 4096, 16, 256, 8
NCORES = 8
CPB = NCORES // B          # cores per batch = 4
NLOC = N // CPB            # points per core = 1024
NPT = 32                   # points per compute tile
RT = NPT * S               # rows per compute tile = 512
NTILES = NLOC // NPT       # 32
CHUNK_ROWS = 2048          # gather chunk (rows)
NCHUNKS = NLOC * S // CHUNK_ROWS
TILES_PER_CHUNK = CHUNK_ROWS // RT
CPAD = 128                 # padded coord row length (fp16 -> 256B)

AO = mybir.AluOpType
AF = mybir.ActivationFunctionType


def _affine(bn_p, lin_b):
    """Fold eval-mode BN (+ preceding linear bias) into scale/bias vectors."""
    bn_p = np.asarray(bn_p, np.float32)
    g, beta, m, v = bn_p[0], bn_p[1], bn_p[2], bn_p[3]
    s = g / np.sqrt(v + EPS)
    t = (np.asarray(lin_b, np.float32) - m) * s + beta
    return s.astype(np.float32), t.astype(np.float32)


def _as_lhst(w):
    """[256, X] -> [128, 2, X] (partition = K within K-tile)."""
    w = np.asarray(w, np.float32)
    return np.ascontiguousarray(w.reshape(2, 128, w.shape[1]).transpose(1, 0, 2))


def _per_part(vec):
    """[256] -> [128, 2]  (channel = j*128 + p)."""
    return np.ascontiguousarray(np.asarray(vec, np.float32).reshape(2, 128).T)


def build_program(ablate=""):
    nc = bacc.Bacc("TRN2", target_bir_lowering=False, debug=False,
                   num_devices=NCORES)

    def din(name, shape, dt):
        return nc.dram_tensor(name, list(shape), dt, kind="ExternalInput")

    featsb = din("featsb", [NLOC, C], BF16)
    posp = din("posp", [NLOC * S, CPAD], FP16)
    fgb = din("fgb", [NLOC * S, C], BF16)
    consts = [
        ("wq3", [128, 2, C], BF16), ("wke3", [128, 2, C], BF16),
        ("wv3", [128, 2, C], BF16),
        ("tkT", [128, 2], F32), ("tvT", [128, 2], F32),
        ("sq", [128, 2], F32), ("tq", [128, 2], F32),
        ("w1m", [4, C], BF16), ("w1b", [4, C], BF16),
        ("sh1m", [128, 2], F32), ("th1m", [128, 2], F32),
        ("sh1b", [128, 2], F32), ("th1b", [128, 2], F32),
        ("w2m", [128, 2, C], BF16), ("w2b", [128, 2, C], BF16),
        ("b2m", [128, 2], F32), ("b2bt", [128, 2], F32),
        ("we1", [128, 2, G], BF16), ("fw", [128, 2, G], BF16),
        ("swe", [G, 1], F32), ("twe", [G, 1], F32),
        ("we2", [G, G], BF16), ("web2", [G, 1], F32),
        ("eoh", [G, 2, 128], BF16), ("eohf", [G, 2, 128], F32),
        ("ident", [128, 128], F32),
    ]
    cdram = {name: din(name, shape, dt) for name, shape, dt in consts}

    out_d = nc.dram_tensor("out", [NLOC, C], F32, kind="ExternalOutput")

    with tile.TileContext(nc) as tc:
        with (
            tc.tile_pool(name="const", bufs=1) as cpool,
            tc.tile_pool(name="big", bufs=1) as bigpool,
            tc.tile_pool(name="gather", bufs=2) as gpool,
            tc.tile_pool(name="work", bufs=2) as wpool,
            tc.tile_pool(name="small", bufs=3) as spool,
            tc.tile_pool(name="psA", bufs=1, space="PSUM") as psA,   # 2 banks
            tc.tile_pool(name="psB", bufs=1, space="PSUM") as psB,   # 2+2 banks
            tc.tile_pool(name="psS", bufs=2, space="PSUM") as psS,   # 2 banks
        ):
            csb = {}
            for name, shape, dt in consts:
                t = cpool.tile(list(shape), dt, tag=name)
                nc.sync.dma_start(t[:], cdram[name][:])
                csb[name] = t

            # ---- phase 1: qT for own quarter -------------------------------
            featsT = bigpool.tile([128, 2, NLOC], BF16, tag="featsT")
            nc.sync.dma_start_transpose(featsT[:, 0, :], featsb[:, 0:128])
            nc.sync.dma_start_transpose(featsT[:, 1, :], featsb[:, 128:256])

            qT = bigpool.tile([128, 2, NLOC], BF16, tag="qT")
            for m in range(2):
                for ch in range(NLOC // 512):
                    ps = psS.tile([128, 512], F32, tag="sm")
                    for kt in range(2):
                        nc.tensor.matmul(
                            ps[:],
                            csb["wq3"][:, kt, m * 128:(m + 1) * 128],
                            featsT[:, kt, ch * 512:(ch + 1) * 512],
                            start=(kt == 0), stop=(kt == 1))
                    nc.scalar.activation(
                        qT[:, m, ch * 512:(ch + 1) * 512], ps[:], AF.Relu,
                        bias=csb["tq"][:, m:m + 1], scale=csb["sq"][:, m:m + 1])

            # ---- phase 2 ------------------------------------------------------
            nchunks = 0 if ablate == "phase1" else NCHUNKS
            for c in range(nchunks):
                r0c = c * CHUNK_ROWS
                # transpose-load gathered feats + pos rows for this chunk
                fgT = gpool.tile([128, 2, CHUNK_ROWS], BF16, tag="fgT")
                nc.sync.dma_start_transpose(
                    fgT[:, 0, :], fgb[r0c:r0c + CHUNK_ROWS, 0:128])
                nc.sync.dma_start_transpose(
                    fgT[:, 1, :], fgb[r0c:r0c + CHUNK_ROWS, 128:256])
                posTc = gpool.tile([128, CHUNK_ROWS], FP16, tag="posTc")
                nc.sync.dma_start_transpose(
                    posTc[:], posp[r0c:r0c + CHUNK_ROWS, :])

                for lt in range(TILES_PER_CHUNK):
                    t = c * TILES_PER_CHUNK + lt
                    r0 = lt * RT
                    pt0 = t * NPT

                    # kg/vg = projections of gathered feature rows (T-layout)
                    kgt = wpool.tile([128, 2, RT], BF16, tag="kgt")
                    vgt = wpool.tile([128, 2, RT], BF16, tag="vgt")
                    for (gt, wname, tname, act) in (
                        (kgt, "wke3", "tkT", AF.Relu),
                        (vgt, "wv3", "tvT", AF.Identity),
                    ):
                        gp = psA.tile([128, 2, RT], F32, tag="hps")
                        for mj in range(2):
                            for kt in range(2):
                                nc.tensor.matmul(
                                    gp[:, mj, :],
                                    csb[wname][:, kt, mj * 128:(mj + 1) * 128],
                                    fgT[:, kt, r0:r0 + RT],
                                    start=(kt == 0), stop=(kt == 1))
                        for mj in range(2):
                            nc.scalar.activation(
                                gt[:, mj, :], gp[:, mj, :], act,
                                bias=csb[tname][:, mj:mj + 1], scale=1.0)

                    h1m = wpool.tile([128, 2, RT], BF16, tag="h1m")
                    h1b = wpool.tile([128, 2, RT], BF16, tag="h1b")
                    for (h1, w1, sh, th) in (
                        (h1m, "w1m", "sh1m", "th1m"),
                        (h1b, "w1b", "sh1b", "th1b"),
                    ):
                        hp = psA.tile([128, 2, RT], F32, tag="hps")
                        for mj in range(2):
                            nc.tensor.matmul(
                                hp[:, mj, :],
                                csb[w1][0:3, mj * 128:(mj + 1) * 128],
                                posTc[0:3, r0:r0 + RT], start=True, stop=True)
                        for mj in range(2):
                            nc.scalar.activation(
                                h1[:, mj, :], hp[:, mj, :], AF.Relu,
                                bias=csb[th][:, mj:mj + 1],
                                scale=csb[sh][:, mj:mj + 1])

                    pem = psB.tile([128, 2, RT], F32, tag="pem")
                    peb = psB.tile([128, 2, RT], F32, tag="peb")
                    for (pp, w2, h1) in ((pem, "w2m", h1m), (peb, "w2b", h1b)):
                        for mj in range(2):
                            for kt in range(2):
                                nc.tensor.matmul(
                                    pp[:, mj, :],
                                    csb[w2][:, kt, mj * 128:(mj + 1) * 128],
                                    h1[:, kt, :],
                                    start=(kt == 0), stop=(kt == 1))

                    t1 = wpool.tile([128, 2, NPT, S], BF16, tag="t1")
                    qb = qT[:, :, pt0:pt0 + NPT].unsqueeze(3) \
                        .broadcast_to((128, 2, NPT, S))
                    nc.vector.tensor_sub(
                        t1[:],
                        kgt[:].rearrange("p j (n s) -> p j n s", s=S),
                        qb)

                    t2 = wpool.tile([128, 2, RT], BF16, tag="t2")
                    for mj in range(2):
                        nc.vector.scalar_tensor_tensor(
                            t2[:, mj, :], pem[:, mj, :],
                            csb["b2m"][:, mj:mj + 1],
                            t1[:, mj, :, :].rearrange("p n s -> p (n s)"),
                            op0=AO.add, op1=AO.mult)

                    lg = psS.tile([G, RT], F32, tag="sm")
                    nc.tensor.matmul(lg[:], csb["we1"][:, 0, :], t2[:, 0, :],
                                     start=True, stop=False)
                    nc.tensor.matmul(lg[:], csb["we1"][:, 1, :], t2[:, 1, :],
                                     start=False, stop=False)
                    nc.tensor.matmul(lg[:], csb["fw"][:, 0, :], h1b[:, 0, :],
                                     start=False, stop=False)
                    nc.tensor.matmul(lg[:], csb["fw"][:, 1, :], h1b[:, 1, :],
                                     start=False, stop=True)
                    hw = spool.tile([G, RT], BF16, tag="hw")
                    nc.scalar.activation(hw[:], lg[:], AF.Relu,
                                         bias=csb["twe"][:],
                                         scale=csb["swe"][:])

                    l2 = psS.tile([G, RT], F32, tag="sm")
                    nc.tensor.matmul(l2[:], csb["we2"][:], hw[:],
                                     start=True, stop=True)

                    e = spool.tile([G, RT], BF16, tag="e")
                    nc.scalar.activation(e[:], l2[:], AF.Exp,
                                         bias=csb["web2"][:], scale=1.0)

                    esum = spool.tile([G, NPT], F32, tag="esum")
                    nc.vector.reduce_sum(
                        esum[:], e[:].rearrange("p (n s) -> p n s", s=S),
                        axis=mybir.AxisListType.X)
                    rinv = spool.tile([G, NPT], F32, tag="rinv")
                    nc.vector.reciprocal(rinv[:], esum[:])

                    eb = psA.tile([128, 2, RT], F32, tag="hps")
                    for mj in range(2):
                        nc.tensor.matmul(eb[:, mj, :], csb["eoh"][:, mj, :],
                                         e[:], start=True, stop=True)

                    val = wpool.tile([128, 2, RT], BF16, tag="val")
                    nc.vector.tensor_add(val[:], vgt[:], peb[:])

                    prod = wpool.tile([128, 2, RT], BF16, tag="prod")
                    nc.vector.tensor_mul(prod[:], val[:], eb[:])
                    outp = spool.tile([128, 2, NPT], F32, tag="outp")
                    nc.vector.reduce_sum(
                        outp[:],
                        prod[:].rearrange("p j (n s) -> p j n s", s=S),
                        axis=mybir.AxisListType.X)

                    rb = psS.tile([128, 2, NPT], F32, tag="sm")
                    for mj in range(2):
                        nc.tensor.matmul(rb[:, mj, :], csb["eohf"][:, mj, :],
                                         rinv[:], start=True, stop=True)
                    outn = spool.tile([128, 2, NPT], F32, tag="outn")
                    nc.vector.tensor_mul(outn[:], outp[:], rb[:])
                    for mj in range(2):
                        nc.vector.tensor_scalar_add(
                            outn[:, mj, :], outn[:, mj, :],
                            csb["b2bt"][:, mj:mj + 1])

                    tr = psS.tile([NPT, 2, 128], F32, tag="sm")
                    for mj in range(2):
                        nc.tensor.transpose(tr[:, mj, :], outn[:, mj, :],
                                            csb["ident"][:])
                    orows = spool.tile([NPT, C], F32, tag="orows")
                    nc.scalar.copy(orows[:],
                                   tr[:].rearrange("p a b -> p (a b)"))
                    nc.sync.dma_start(out_d[pt0:pt0 + NPT, :], orows[:])

    nc.compile()
    return nc


def host_prep(inputs):
    """Fold BN, cast/transpose weights, build per-core input maps."""
    f = {k: np.asarray(v) for k, v in inputs.items()}
    feats, coords, index = f["feats"], f["coords"], f["index"]
    index = index.astype(np.int64)

    s_q, t_q = _affine(f["bnq"], f["bq"])
    s_k, t_k = _affine(f["bnk"], f["bk"])
    s_hm, t_hm = _affine(f["pm_bn"], f["pm_b1"])
    s_hb, t_hb = _affine(f["pb_bn"], f["pb_b1"])

    b2b_we = np.asarray(f["pb_b2"], np.float32) @ np.asarray(f["we_w1"], np.float32)
    s_we, t_we = _affine(f["we_bn"], np.asarray(f["we_b1"], np.float32) + b2b_we)

    wk_eff = np.asarray(f["wk"], np.float32) * s_k[None, :]
    F_mat = np.asarray(f["pb_w2"], np.float32) @ np.asarray(f["we_w1"], np.float32)

    eoh = np.zeros((G, 2, 128), np.float32)
    for g in range(G):
        j, p0 = divmod(g * 32, 128)
        eoh[g, j, p0:p0 + 32] = 1.0

    shared = {
        "wq3": _as_lhst(f["wq"]).astype(NP_BF16),
        "wke3": _as_lhst(wk_eff).astype(NP_BF16),
        "wv3": _as_lhst(f["wv"]).astype(NP_BF16),
        "tkT": _per_part(t_k),
        "tvT": _per_part(np.asarray(f["bv"], np.float32)),
        "sq": _per_part(s_q), "tq": _per_part(t_q),
        "w1m": np.concatenate([np.asarray(f["pm_w1"], np.float32),
                               np.zeros((1, C), np.float32)], 0).astype(NP_BF16),
        "w1b": np.concatenate([np.asarray(f["pb_w1"], np.float32),
                               np.zeros((1, C), np.float32)], 0).astype(NP_BF16),
        "sh1m": _per_part(s_hm), "th1m": _per_part(t_hm),
        "sh1b": _per_part(s_hb), "th1b": _per_part(t_hb),
        "w2m": _as_lhst(f["pm_w2"]).astype(NP_BF16),
        "w2b": _as_lhst(f["pb_w2"]).astype(NP_BF16),
        "b2m": _per_part(f["pm_b2"]),
        "b2bt": _per_part(f["pb_b2"]),
        "we1": _as_lhst(f["we_w1"]).astype(NP_BF16),
        "fw": _as_lhst(F_mat).astype(NP_BF16),
        "swe": s_we.reshape(G, 1), "twe": t_we.reshape(G, 1),
        "we2": np.asarray(f["we_w2"], np.float32).astype(NP_BF16),
        "web2": np.asarray(f["we_b2"], np.float32).reshape(G, 1),
        "eoh": eoh.astype(NP_BF16),
        "eohf": eoh,
        "ident": np.eye(128, dtype=np.float32),
    }

    in_maps = []
    for core in range(NCORES):
        b, qc = divmod(core, CPB)
        qoff = qc * NLOC
        fb32 = np.asarray(feats[b], np.float32)
        featsb = fb32[qoff:qoff + NLOC].astype(NP_BF16)
        idx = index[b, qoff:qoff + NLOC, :].reshape(-1)
        fgb = fb32[idx].astype(NP_BF16)                    # gathered feat rows
        cb = np.asarray(coords[b], np.float32)
        pos = cb[qoff:qoff + NLOC][:, None, :] - cb[idx.reshape(NLOC, S)]
        posp = np.zeros((NLOC * S, CPAD), np.float16)
        posp[:, :3] = pos.reshape(NLOC * S, 3)
        m = dict(shared)
        m["featsb"] = featsb
        m["posp"] = posp
        m["fgb"] = fgb
        in_maps.append(m)
    return in_maps


_NC_CACHE = {}


def _get_program():
    ablate = os.environ.get("KERNEL_ABLATE", "")
    key = "nc" + ablate
    if key not in _NC_CACHE:
        _NC_CACHE[key] = build_program(ablate)
    return _NC_CACHE[key]


def kernel(**inputs):
    nc = _get_program()
    in_maps = host_prep(inputs)
    res = bass_utils.run_bass_kernel_spmd(
        nc, in_maps, list(range(NCORES)),
        trace=bool(int(os.environ.get("KERNEL_TRACE", "0"))))
    _NC_CACHE["last_results"] = res
    out = np.zeros((B, N, C), np.float32)
    for core in range(NCORES):
        b, qc = divmod(core, CPB)
        out[b, qc * NLOC:(qc + 1) * NLOC, :] = res.results[core]["out"]
    return out



# revision 21
# speedup vs baseline: 1.5486x; 1.5486x over previous
"""Trainium2 Bass kernel for GroupedVectorSA (gnn message passing), v2.

Self-contained: accepts FULL inputs (as produced by setup_inputs()), shards
across 8 NeuronCores (batch b = core//4, quarter of N = core%4), runs one
SPMD Bass/Tile program via bass_utils.run_bass_kernel_spmd, reassembles the
full [B, N, C] output.

v2 design notes (vs v1 baseline ~700us):
  - All K=256 projections run as fp8e4(e4m3) DoubleRow matmuls (0.5 cyc/row).
  - Host pre-transposes every operand (no on-device DMA transposes) and
    pre-gathers neighbor feature rows (input-only work).
  - Linear biases enter PSUM via K=1 ones-row matmuls; BN affines fold into
    weights host-side; per-matrix power-of-2 scales keep fp8 operands in
    range and are exactly compensated downstream.
  - relu(kg')-q fused into one DVE scalar_tensor_tensor from PSUM.
  - val = vg + peb accumulated inside PSUM by the PE (no vector add).
  - softmax: unnormalized e drives the weighted sum; the denominator
    (esum) ships to HBM and the host divides during the unshard, along
    with the constant output bias (pb_b2 + bv).
  - PSUM choreography: one rotating 3-buffer [128,1024] tag for all
    short-lived psum tensors + 1 buffer for val (lives across the softmax);
    prod/outp run software-pipelined one tile behind so the in-order DVE
    queue (t1, t2, prod) never stalls.
  - Engine balance: Scalar h1 acts + hw/exp; DVE t1/t2/prod; Pool esum +
    S-window output reduce.
"""

import os
import sys

import numpy as np

try:
    import concourse  # noqa: F401
except ImportError:
    sys.path.insert(0, "/opt/trn_rl_repo")

import ml_dtypes

import concourse.bacc as bacc
import concourse.bass as bass  # noqa: F401
import concourse.mybir as mybir
import concourse.tile as tile
from concourse import bass_utils

F32 = mybir.dt.float32
BF16 = mybir.dt.bfloat16
FP16 = mybir.dt.float16
FP8 = mybir.dt.float8e4

NP_BF16 = ml_dtypes.bfloat16
NP_FP8 = ml_dtypes.float8_e4m3fn

EPS = 1e-5
B, N, S, C, G = 2, 4096, 16, 256, 8
NCORES = 8
CPB = NCORES // B          # cores per batch = 4
NLOC = N // CPB            # points per core = 1024
NPT = 32                   # points per compute tile
RT = NPT * S               # gathered rows per compute tile = 512
NTILES = NLOC // NPT       # 32
NCHUNK = 4                 # fgT8 DMA chunks
TPC = NTILES // NCHUNK     # tiles per chunk = 8
CCOLS = NLOC * S // NCHUNK  # columns per chunk = 4096

# power-of-2 fp8 range scales (exactly compensated downstream)
S_QW = 8.0     # wq
S_KW = 4.0     # wk_eff; kg-psum, t1, qm carry S_KW
S_W2M = 2.0    # pm_w2; pem-psum carries S_W2M -> t2 carries S_KW*S_W2M=8
S_WE1 = 16.0   # we_w1; lg carries S_WE1*8 = 128
S_FW = 128.0   # F = pb_w2 @ we_w1 (must equal S_WE1*S_KW*S_W2M)
S_VAL = 8.0    # wv and pb_w2 (val-psum, outacc carry S_VAL)
S_W1 = 4.0     # pm_w1/pb_w1 fp8 lhsT scale (h1 act divides back)

AO = mybir.AluOpType
AF = mybir.ActivationFunctionType
AX = mybir.AxisListType
DR = mybir.MatmulPerfMode.DoubleRow


def _affine(bn_p, lin_b):
    """Fold eval-mode BN (+ preceding linear bias) into scale/bias vectors."""
    bn_p = np.asarray(bn_p, np.float32)
    g, beta, m, v = bn_p[0], bn_p[1], bn_p[2], bn_p[3]
    s = g / np.sqrt(v + EPS)
    t = (np.asarray(lin_b, np.float32) - m) * s + beta
    return s.astype(np.float32), t.astype(np.float32)


def _as_lhst(w):
    """[256, X] -> [128, 2, X] (partition p, k-half j: k = j*128 + p)."""
    w = np.asarray(w, np.float32)
    return np.ascontiguousarray(w.reshape(2, 128, w.shape[1]).transpose(1, 0, 2))


def _per_part(vec):
    """[256] -> [128, 2]  (channel = j*128 + p)."""
    return np.ascontiguousarray(np.asarray(vec, np.float32).reshape(2, 128).T)


def build_program(h1_fp8=True):
    nc = bacc.Bacc("TRN2", target_bir_lowering=False, debug=False,
                   num_devices=NCORES)

    def din(name, shape, dt):
        return nc.dram_tensor(name, list(shape), dt, kind="ExternalInput")

    featsT8 = din("featsT8", [128, 2, NLOC], FP8)
    fgb8 = din("fgb8", [128, 2, NLOC * S], FP8)
    if h1_fp8:
        pos_d = din("pos8", [2, 2, NLOC * S], FP8)
    else:
        pos_d = din("pos4", [4, NLOC * S], BF16)
    consts = [
        ("wq8", [128, 2, C], FP8), ("wke8", [128, 2, C], FP8),
        ("wv8", [128, 2, C], FP8),
        ("w2m8", [128, 2, C], FP8), ("w2b8", [128, 2, C], FP8),
        ("dw2b8", [128, 2, C], FP8),
        ("we18", [128, 2, 2 * G], FP8), ("fw8", [128, 2, 2 * G], FP8),
        ("sqv", [128, 2], F32), ("tqv", [128, 2], F32),
        ("tkrow", [1, C], BF16), ("b2row", [1, C], BF16),
        ("ones", [1, RT], BF16),
        ("swe", [G, 1], F32), ("twe", [G, 1], F32),
        ("we2", [G, G], BF16), ("web2", [G, 1], F32),
        ("eoh", [G, 2, 128], BF16),
    ]
    if h1_fp8:
        consts += [("w1m8", [2, 2, C], FP8), ("w1b8", [2, 2, C], FP8)]
    else:
        consts += [("w1m", [4, C], BF16), ("w1b", [4, C], BF16)]
    cdram = {name: din(name, shape, dt) for name, shape, dt in consts}

    out_d = nc.dram_tensor("out", [128, NTILES, 2, NPT], F32,
                           kind="ExternalOutput")
    esum_d = nc.dram_tensor("esum", [G, NLOC], F32, kind="ExternalOutput")

    with tile.TileContext(nc) as tc:
        with (
            tc.tile_pool(name="const", bufs=1) as cpool,
            tc.tile_pool(name="big", bufs=1) as bigpool,
            tc.tile_pool(name="work", bufs=2) as wpool,
            tc.tile_pool(name="small", bufs=3) as spool,
            tc.tile_pool(name="ps", bufs=3, space="PSUM") as psP,
        ):
            csb = {}
            for name, shape, dt in consts:
                t = cpool.tile(list(shape), dt, tag=name)
                nc.sync.dma_start(t[:], cdram[name][:])
                csb[name] = t

            featsT = cpool.tile([128, 2, NLOC], FP8, tag="featsT")
            nc.sync.dma_start(featsT[:], featsT8[:])
            posT = cpool.tile(
                [2, 2, NLOC * S] if h1_fp8 else [4, NLOC * S],
                FP8 if h1_fp8 else BF16, tag="posT")
            nc.sync.dma_start(posT[:], pos_d[:])
            fgc = []
            for cch in range(NCHUNK):
                t = cpool.tile([128, 2, CCOLS], FP8, tag=f"fg{cch}")
                nc.sync.dma_start(t[:], fgb8[:, :, cch * CCOLS:(cch + 1) * CCOLS])
                fgc.append(t)

            # tile-local layout [p, tile, j, n] so (j, n) flattens contiguous
            outacc = bigpool.tile([128, NTILES, 2, NPT], F32, tag="outacc")
            esumacc = bigpool.tile([G, NLOC], F32, tag="esumacc")

            # ---- q phase: qm = S_KW * relu(bn_q(feats @ wq + bq)) ----------
            # tile-local layout [p, tile, j, n] so (j, n) flattens contiguous
            qm = bigpool.tile([128, NTILES, 2, NPT], BF16, tag="qm")
            tpch = 512 // NPT  # tiles per 512-point chunk
            for ch in range(NLOC // 512):
                pq = psP.tile([128, 1024], F32, tag="rot")
                for mj in range(2):
                    nc.tensor.matmul(
                        pq[:, mj * 512:(mj + 1) * 512],
                        csb["wq8"][:, :, mj * 128:(mj + 1) * 128],
                        featsT[:, :, ch * 512:(ch + 1) * 512],
                        start=True, stop=True, perf_mode=DR)
                for mj in range(2):
                    nc.scalar.activation(
                        qm[:, ch * tpch:(ch + 1) * tpch, mj, :],
                        pq[:, mj * 512:(mj + 1) * 512]
                            .rearrange("p (t n) -> p t n", n=NPT),
                        AF.Relu,
                        bias=csb["tqv"][:, mj:mj + 1],
                        scale=csb["sqv"][:, mj:mj + 1])

            # ---- main tile loop (prod/outp run one tile behind) ------------
            prev = None  # (pv, pe, pt0) of previous tile

            def emit_prod(pv, ebb, ti):
                # prod = val'(PSUM) * ebb(SBUF)  on DVE
                prod = wpool.tile([128, 64, S], BF16, tag="prod")
                nc.vector.scalar_tensor_tensor(
                    prod[:], pv[:].rearrange("p (m s) -> p m s", s=S),
                    0.0, ebb[:].rearrange("p (m s) -> p m s", s=S),
                    op0=AO.add, op1=AO.mult)
                # S-window reduce: pool halving adds, then tiny DVE reduce
                p8 = wpool.tile([128, 64, S // 2], F32, tag="p8")
                nc.gpsimd.tensor_add(p8[:], prod[:, :, 0:8], prod[:, :, 8:16])
                p4 = wpool.tile([128, 64, S // 4], F32, tag="p4")
                nc.gpsimd.tensor_add(p4[:], p8[:, :, 0:4], p8[:, :, 4:8])
                p2 = wpool.tile([128, 64, S // 8], F32, tag="p2")
                nc.gpsimd.tensor_add(p2[:], p4[:, :, 0:2], p4[:, :, 2:4])
                nc.vector.reduce_sum(
                    outacc[:, ti, :, :].rearrange("p j n -> p (j n)"),
                    p2[:], axis=AX.X)

            for t in range(NTILES):
                pt0 = t * NPT
                fg = fgc[t // TPC]
                r0 = (t % TPC) * RT
                g0 = t * RT  # global row offset for pos

                # pos-path hidden layers; ph_mj = [h1m_mj | h1b_mj]
                h18 = wpool.tile([128, 2, 2, 512], FP8, tag="h18")
                for mj in range(2):
                    ph = psP.tile([128, 1024], F32, tag="rot")
                    for mlp, wkey in ((0, "m"), (1, "b")):
                        if h1_fp8:
                            nc.tensor.matmul(
                                ph[:, mlp * 512:(mlp + 1) * 512],
                                csb[f"w1{wkey}8"][:, :, mj * 128:(mj + 1) * 128],
                                posT[:, :, g0:g0 + RT],
                                start=True, stop=True, perf_mode=DR)
                        else:
                            nc.tensor.matmul(
                                ph[:, mlp * 512:(mlp + 1) * 512],
                                csb[f"w1{wkey}"][:, mj * 128:(mj + 1) * 128],
                                posT[:, g0:g0 + RT],
                                start=True, stop=True)
                    nc.scalar.activation(
                        h18[:, :, mj, :], ph[:].rearrange("p (l n) -> p l n", l=2),
                        AF.Relu, scale=1.0 / S_W1 if h1_fp8 else 1.0)
                h1m8 = h18[:, 0, :, :]
                h1b8 = h18[:, 1, :, :]

                # kg' = S_KW*(wk_eff @ fg + tk); t1 = relu(kg') - qm
                pk = psP.tile([128, 1024], F32, tag="rot")
                for mj in range(2):
                    nc.tensor.matmul(
                        pk[:, mj * 512:(mj + 1) * 512],
                        csb["wke8"][:, :, mj * 128:(mj + 1) * 128],
                        fg[:, :, r0:r0 + RT],
                        start=True, stop=False, perf_mode=DR)
                    nc.tensor.matmul(
                        pk[:, mj * 512:(mj + 1) * 512],
                        csb["tkrow"][:, mj * 128:(mj + 1) * 128],
                        csb["ones"][:],
                        start=False, stop=True)
                t1 = wpool.tile([128, 64, S], BF16, tag="t1")
                qb = qm[:, t, :, :].rearrange("p j n -> p (j n)") \
                    .unsqueeze(2).broadcast_to((128, 64, S))
                nc.vector.scalar_tensor_tensor(
                    t1[:], pk[:].rearrange("p (m s) -> p m s", s=S),
                    0.0, qb, op0=AO.max, op1=AO.subtract)

                # pem' = S_W2M*(pem + b2m); t2 = pem' * t1 (carries 32x)
                pp = psP.tile([128, 1024], F32, tag="rot")
                for mj in range(2):
                    nc.tensor.matmul(
                        pp[:, mj * 512:(mj + 1) * 512],
                        csb["w2m8"][:, :, mj * 128:(mj + 1) * 128],
                        h1m8,
                        start=True, stop=False, perf_mode=DR)
                    nc.tensor.matmul(
                        pp[:, mj * 512:(mj + 1) * 512],
                        csb["b2row"][:, mj * 128:(mj + 1) * 128],
                        csb["ones"][:],
                        start=False, stop=True)
                t28 = wpool.tile([128, 2, 512], FP8, tag="t28")
                nc.vector.scalar_tensor_tensor(
                    t28[:], pp[:].rearrange("p (j n) -> p j n", j=2),
                    0.0, t1[:].rearrange("p m s -> p (m s)")
                        .rearrange("p (j n) -> p j n", j=2),
                    op0=AO.add, op1=AO.mult)

                # val' = S_VAL*(vg + peb0): both accumulated in PSUM
                pv = psP.tile([128, 1024], F32, tag="pv", bufs=1)
                for mj in range(2):
                    nc.tensor.matmul(
                        pv[:, mj * 512:(mj + 1) * 512],
                        csb["wv8"][:, :, mj * 128:(mj + 1) * 128],
                        fg[:, :, r0:r0 + RT],
                        start=True, stop=False, perf_mode=DR)
                    nc.tensor.matmul(
                        pv[:, mj * 512:(mj + 1) * 512],
                        csb["w2b8"][:, :, mj * 128:(mj + 1) * 128],
                        h1b8,
                        start=False, stop=False, perf_mode=DR)
                    nc.tensor.matmul(
                        pv[:, mj * 512:(mj + 1) * 512],
                        csb["dw2b8"][:, :, mj * 128:(mj + 1) * 128],
                        h1b8,
                        start=False, stop=True, perf_mode=DR)

                # logits: lg' = 256*lg = we18^T t28 + fw8^T h1b8
                pl = psP.tile([2 * G, RT], F32, tag="rot")
                nc.tensor.matmul(pl[:], csb["we18"][:], t28[:],
                                 start=True, stop=False, perf_mode=DR)
                nc.tensor.matmul(pl[:], csb["fw8"][:], h1b8,
                                 start=False, stop=True, perf_mode=DR)
                hw = spool.tile([G, RT], BF16, tag="hw")
                nc.scalar.activation(hw[:], pl[0:G, :], AF.Relu,
                                     bias=csb["twe"][:], scale=csb["swe"][:])
                pl2 = psP.tile([G, RT], F32, tag="rot")
                nc.tensor.matmul(pl2[:], csb["we2"][:], hw[:],
                                 start=True, stop=True)
                e = spool.tile([G, RT], BF16, tag="e")
                nc.scalar.activation(e[:], pl2[:], AF.Exp,
                                     bias=csb["web2"][:], scale=1.0)
                e8 = spool.tile([G, NPT, S // 2], F32, tag="e8")
                ev = e[:].rearrange("p (n s) -> p n s", s=S)
                nc.gpsimd.tensor_add(e8[:], ev[:, :, 0:8], ev[:, :, 8:16])
                e4 = spool.tile([G, NPT, S // 4], F32, tag="e4")
                nc.gpsimd.tensor_add(e4[:], e8[:, :, 0:4], e8[:, :, 4:8])
                nc.vector.reduce_sum(
                    esumacc[:, pt0:pt0 + NPT], e4[:], axis=AX.X)

                # expand e over channel groups; evacuate to SBUF on scalar
                pe = psP.tile([128, 1024], F32, tag="rot")
                for mj in range(2):
                    nc.tensor.matmul(
                        pe[:, mj * 512:(mj + 1) * 512],
                        csb["eoh"][:, mj, :], e[:],
                        start=True, stop=True)
                ebb = wpool.tile([128, 1024], BF16, tag="ebb")
                nc.scalar.copy(ebb[:], pe[:])

                # previous tile's prod/outp (keeps DVE queue stall-free)
                if prev is not None:
                    emit_prod(*prev)
                prev = (pv, ebb, t)

            emit_prod(*prev)

            nc.sync.dma_start(out_d[:], outacc[:])
            nc.sync.dma_start(esum_d[:], esumacc[:])

    nc.compile()
    return nc


def host_prep(inputs, h1_fp8=True):
    """Fold BN, scale/cast weights to fp8, build per-core input maps."""
    f = {k: np.asarray(v) for k, v in inputs.items()}
    feats, coords, index = f["feats"], f["coords"], f["index"]
    index = index.astype(np.int64)

    s_q, t_q = _affine(f["bnq"], f["bq"])
    s_k, t_k = _affine(f["bnk"], f["bk"])
    s_hm, t_hm = _affine(f["pm_bn"], f["pm_b1"])
    s_hb, t_hb = _affine(f["pb_bn"], f["pb_b1"])

    b2b_we = np.asarray(f["pb_b2"], np.float32) @ np.asarray(f["we_w1"], np.float32)
    s_we, t_we = _affine(f["we_bn"], np.asarray(f["we_b1"], np.float32) + b2b_we)

    wk_eff = np.asarray(f["wk"], np.float32) * s_k[None, :]
    F_mat = np.asarray(f["pb_w2"], np.float32) @ np.asarray(f["we_w1"], np.float32)

    # w1' = w1 * bn_scale with ones-row bias fold (pos row 3 == 1)
    def w1_fold(w1, s_h, t_h):
        w = np.asarray(w1, np.float32) * s_h[None, :]
        return np.concatenate([w, t_h[None, :]], 0)  # [4, C]

    w1m_f = w1_fold(f["pm_w1"], s_hm, t_hm)
    w1b_f = w1_fold(f["pb_w1"], s_hb, t_hb)

    eoh = np.zeros((G, 2, 128), np.float32)
    for g in range(G):
        j, p0 = divmod(g * 32, 128)
        eoh[g, j, p0:p0 + 32] = 1.0

    shared = {
        "wq8": (_as_lhst(f["wq"]) * S_QW).astype(NP_FP8),
        "wke8": (_as_lhst(wk_eff) * S_KW).astype(NP_FP8),
        "wv8": (_as_lhst(f["wv"]) * S_VAL).astype(NP_FP8),
        "w2m8": (_as_lhst(f["pm_w2"]) * S_W2M).astype(NP_FP8),
        "w2b8": (_as_lhst(f["pb_w2"]) * S_VAL).astype(NP_FP8),
        "dw2b8": (_as_lhst(f["pb_w2"]) * S_VAL
                  - (_as_lhst(f["pb_w2"]) * S_VAL).astype(NP_FP8)
                  .astype(np.float32)).astype(NP_FP8),
        "we18": np.concatenate(
            [(_as_lhst(f["we_w1"]) * S_WE1), np.zeros((128, 2, G), np.float32)],
            axis=2).astype(NP_FP8),
        "fw8": np.concatenate(
            [(_as_lhst(F_mat) * S_FW), np.zeros((128, 2, G), np.float32)],
            axis=2).astype(NP_FP8),
        "sqv": _per_part(s_q * S_KW / S_QW),
        "tqv": _per_part(t_q * S_KW),
        "tkrow": (t_k * S_KW).astype(NP_BF16).reshape(1, C),
        "b2row": (np.asarray(f["pm_b2"], np.float32) * S_W2M)
            .astype(NP_BF16).reshape(1, C),
        "ones": np.ones((1, RT), NP_BF16),
        "swe": (s_we / (S_WE1 * S_KW * S_W2M)).reshape(G, 1).astype(np.float32),
        "twe": t_we.reshape(G, 1).astype(np.float32),
        "we2": np.asarray(f["we_w2"], np.float32).astype(NP_BF16),
        "web2": np.asarray(f["we_b2"], np.float32).reshape(G, 1),
        "eoh": eoh.astype(NP_BF16),
    }
    if h1_fp8:
        # k = i*2 + p mapping for [2, 2, C] lhsT / [2, 2, cols] rhs
        def pack22(w4):  # [4, C] -> [2, 2, C]
            return np.ascontiguousarray(
                w4.reshape(2, 2, -1).transpose(1, 0, 2))
        shared["w1m8"] = (pack22(w1m_f) * S_W1).astype(NP_FP8)
        shared["w1b8"] = (pack22(w1b_f) * S_W1).astype(NP_FP8)
    else:
        shared["w1m"] = w1m_f.astype(NP_BF16)
        shared["w1b"] = w1b_f.astype(NP_BF16)

    in_maps = []
    for core in range(NCORES):
        b, qc = divmod(core, CPB)
        qoff = qc * NLOC
        fb32 = np.asarray(feats[b], np.float32)
        # featsT8: [128, 2, NLOC], [p, j, n] = feats[n, j*128+p]
        fq = fb32[qoff:qoff + NLOC]
        featsT = np.ascontiguousarray(
            fq.T.reshape(2, 128, NLOC).transpose(1, 0, 2)).astype(NP_FP8)
        idx = index[b, qoff:qoff + NLOC, :].reshape(-1)
        fg = fb32[idx]                                   # [NLOC*S, C]
        fgb8 = np.ascontiguousarray(
            fg.T.reshape(2, 128, NLOC * S).transpose(1, 0, 2)).astype(NP_FP8)
        cb = np.asarray(coords[b], np.float32)
        pos = cb[qoff:qoff + NLOC][:, None, :] - cb[idx.reshape(NLOC, S)]
        pos4 = np.concatenate(
            [pos.reshape(NLOC * S, 3).T,
             np.ones((1, NLOC * S), np.float32)], 0)     # [4, NLOC*S]
        m = dict(shared)
        m["featsT8"] = featsT
        m["fgb8"] = fgb8
        if h1_fp8:
            m["pos8"] = np.ascontiguousarray(
                pos4.reshape(2, 2, NLOC * S).transpose(1, 0, 2)).astype(NP_FP8)
        else:
            m["pos4"] = pos4.astype(NP_BF16)
        in_maps.append(m)

    # host-side unshard constants
    bias = (np.asarray(f["pb_b2"], np.float32)
            + np.asarray(f["bv"], np.float32))           # [C]
    return in_maps, bias


_NC_CACHE = {}


def _h1_fp8():
    return os.environ.get("KERNEL_H1BF16", "1") != "1"


def _get_program():
    key = "nc" + ("8" if _h1_fp8() else "16")
    if key not in _NC_CACHE:
        _NC_CACHE[key] = build_program(h1_fp8=_h1_fp8())
    return _NC_CACHE[key]


def unshard(results, bias):
    out = np.zeros((B, N, C), np.float32)
    for core in range(NCORES):
        b, qc = divmod(core, CPB)
        o = np.asarray(results[core]["out"], np.float32)    # [128, T, 2, n]
        es = np.asarray(results[core]["esum"], np.float32)  # [G, NLOC]
        # channel c = j*128 + p ; group g = c // 32 ; point = t*NPT + n
        oc = o.transpose(1, 3, 2, 0).reshape(NLOC, C)       # [n, c]
        denom = np.repeat(es.T, C // G, axis=1) * S_VAL     # [n, c]
        out[b, qc * NLOC:(qc + 1) * NLOC] = oc / denom + bias[None, :]
    return out


def kernel(**inputs):
    nc = _get_program()
    in_maps, bias = host_prep(inputs, h1_fp8=_h1_fp8())
    res = bass_utils.run_bass_kernel_spmd(
        nc, in_maps, list(range(NCORES)),
        trace=bool(int(os.environ.get("KERNEL_TRACE", "0"))))
    _NC_CACHE["last_results"] = res
    return unshard(res.results, bias)


# revision 22
# speedup vs baseline: 1.6128x; 1.0415x over previous
"""Trainium2 Bass kernel for GroupedVectorSA (gnn message passing), v2.

Self-contained: accepts FULL inputs (as produced by setup_inputs()), shards
across 8 NeuronCores (batch b = core//4, quarter of N = core%4), runs one
SPMD Bass/Tile program via bass_utils.run_bass_kernel_spmd, reassembles the
full [B, N, C] output.

v2 design notes (vs v1 baseline ~700us):
  - All K=256 projections run as fp8e4(e4m3) DoubleRow matmuls (0.5 cyc/row).
  - Host pre-transposes every operand (no on-device DMA transposes) and
    pre-gathers neighbor feature rows (input-only work).
  - Linear biases enter PSUM via K=1 ones-row matmuls; BN affines fold into
    weights host-side; per-matrix power-of-2 scales keep fp8 operands in
    range and are exactly compensated downstream.
  - relu(kg')-q fused into one DVE scalar_tensor_tensor from PSUM.
  - val = vg + peb accumulated inside PSUM by the PE (no vector add).
  - softmax: unnormalized e drives the weighted sum; the denominator
    (esum) ships to HBM and the host divides during the unshard, along
    with the constant output bias (pb_b2 + bv).
  - PSUM choreography: one rotating 3-buffer [128,1024] tag for all
    short-lived psum tensors + 1 buffer for val (lives across the softmax);
    prod/outp run software-pipelined one tile behind so the in-order DVE
    queue (t1, t2, prod) never stalls.
  - Engine balance: Scalar h1 acts + hw/exp; DVE t1/t2/prod; Pool esum +
    S-window output reduce.
"""

import os
import sys

import numpy as np

try:
    import concourse  # noqa: F401
except ImportError:
    sys.path.insert(0, "/opt/trn_rl_repo")

import ml_dtypes

import concourse.bacc as bacc
import concourse.bass as bass  # noqa: F401
import concourse.mybir as mybir
import concourse.tile as tile
from concourse import bass_utils

F32 = mybir.dt.float32
BF16 = mybir.dt.bfloat16
FP16 = mybir.dt.float16
FP8 = mybir.dt.float8e4

NP_BF16 = ml_dtypes.bfloat16
NP_FP8 = ml_dtypes.float8_e4m3fn

EPS = 1e-5
B, N, S, C, G = 2, 4096, 16, 256, 8
NCORES = 8
CPB = NCORES // B          # cores per batch = 4
NLOC = N // CPB            # points per core = 1024
NPT = 32                   # points per compute tile
RT = NPT * S               # gathered rows per compute tile = 512
NTILES = NLOC // NPT       # 32
NCHUNK = 4                 # fgT8 DMA chunks
TPC = NTILES // NCHUNK     # tiles per chunk = 8
CCOLS = NLOC * S // NCHUNK  # columns per chunk = 4096

# power-of-2 fp8 range scales (exactly compensated downstream)
S_QW = 8.0     # wq
S_KW = 4.0     # wk_eff; kg-psum, t1, qm carry S_KW
S_W2M = 2.0    # pm_w2; pem-psum carries S_W2M -> t2 carries S_KW*S_W2M=8
S_WE1 = 16.0   # we_w1; lg carries S_WE1*8 = 128
S_FW = 128.0   # F = pb_w2 @ we_w1 (must equal S_WE1*S_KW*S_W2M)
S_VAL = 8.0    # wv and pb_w2 (val-psum, outacc carry S_VAL)
S_W1 = 4.0     # pm_w1/pb_w1 fp8 lhsT scale (h1 act divides back)

AO = mybir.AluOpType
AF = mybir.ActivationFunctionType
AX = mybir.AxisListType
DR = mybir.MatmulPerfMode.DoubleRow


def _affine(bn_p, lin_b):
    """Fold eval-mode BN (+ preceding linear bias) into scale/bias vectors."""
    bn_p = np.asarray(bn_p, np.float32)
    g, beta, m, v = bn_p[0], bn_p[1], bn_p[2], bn_p[3]
    s = g / np.sqrt(v + EPS)
    t = (np.asarray(lin_b, np.float32) - m) * s + beta
    return s.astype(np.float32), t.astype(np.float32)


def _as_lhst(w):
    """[256, X] -> [128, 2, X] (partition p, k-half j: k = j*128 + p)."""
    w = np.asarray(w, np.float32)
    return np.ascontiguousarray(w.reshape(2, 128, w.shape[1]).transpose(1, 0, 2))


def _per_part(vec):
    """[256] -> [128, 2]  (channel = j*128 + p)."""
    return np.ascontiguousarray(np.asarray(vec, np.float32).reshape(2, 128).T)


def build_program(h1_fp8=True):
    nc = bacc.Bacc("TRN2", target_bir_lowering=False, debug=False,
                   num_devices=NCORES)

    def din(name, shape, dt):
        return nc.dram_tensor(name, list(shape), dt, kind="ExternalInput")

    featsT8 = din("featsT8", [128, 2, NLOC], FP8)
    fgb8 = din("fgb8", [128, 2, NLOC * S], FP8)
    if h1_fp8:
        pos_d = din("pos8", [2, 2, NLOC * S], FP8)
    else:
        pos_d = din("pos4", [4, NLOC * S], BF16)
    consts = [
        ("wq8", [128, 2, C], FP8), ("wke8", [128, 2, C], FP8),
        ("wv8", [128, 2, C], FP8),
        ("w2m8", [128, 2, C], FP8), ("w2b8", [128, 2, C], FP8),
        ("dw2b8", [128, 2, C], FP8),
        ("we18", [128, 2, 2 * G], FP8), ("fw8", [128, 2, 2 * G], FP8),
        ("sqv", [128, 2], F32), ("tqv", [128, 2], F32),
        ("tkv", [128, 2], F32), ("b2v", [128, 2], F32),
        ("swe", [G, 1], F32), ("twe", [G, 1], F32),
        ("we2", [G, G], BF16), ("web2", [G, 1], F32),
        ("eoh", [G, 2, 128], BF16),
    ]
    if h1_fp8:
        consts += [("w1m8", [2, 2, C], FP8), ("w1b8", [2, 2, C], FP8)]
    else:
        consts += [("w1m", [4, C], BF16), ("w1b", [4, C], BF16)]
    cdram = {name: din(name, shape, dt) for name, shape, dt in consts}

    out_d = nc.dram_tensor("out", [128, NTILES, 2, NPT], F32,
                           kind="ExternalOutput")
    esum_d = nc.dram_tensor("esum", [G, NLOC], F32, kind="ExternalOutput")

    with tile.TileContext(nc) as tc:
        with (
            tc.tile_pool(name="const", bufs=1) as cpool,
            tc.tile_pool(name="big", bufs=1) as bigpool,
            tc.tile_pool(name="work", bufs=2) as wpool,
            tc.tile_pool(name="small", bufs=3) as spool,
            tc.tile_pool(name="ps", bufs=3, space="PSUM") as psP,
        ):
            csb = {}
            for name, shape, dt in consts:
                t = cpool.tile(list(shape), dt, tag=name)
                nc.sync.dma_start(t[:], cdram[name][:])
                csb[name] = t

            featsT = cpool.tile([128, 2, NLOC], FP8, tag="featsT")
            nc.sync.dma_start(featsT[:], featsT8[:])
            posT = cpool.tile(
                [2, 2, NLOC * S] if h1_fp8 else [4, NLOC * S],
                FP8 if h1_fp8 else BF16, tag="posT")
            nc.sync.dma_start(posT[:], pos_d[:])
            fgc = []
            for cch in range(NCHUNK):
                t = cpool.tile([128, 2, CCOLS], FP8, tag=f"fg{cch}")
                nc.sync.dma_start(t[:], fgb8[:, :, cch * CCOLS:(cch + 1) * CCOLS])
                fgc.append(t)

            # tile-local layout [p, tile, j, n] so (j, n) flattens contiguous
            outacc = bigpool.tile([128, NTILES, 2, NPT], F32, tag="outacc")
            esumacc = bigpool.tile([G, NLOC], F32, tag="esumacc")

            # ---- q phase: qm = S_KW * relu(bn_q(feats @ wq + bq)) ----------
            # tile-local layout [p, tile, j, n] so (j, n) flattens contiguous
            qm = bigpool.tile([128, NTILES, 2, NPT], BF16, tag="qm")
            tpch = 512 // NPT  # tiles per 512-point chunk
            for ch in range(NLOC // 512):
                pq = psP.tile([128, 1024], F32, tag="rot")
                for mj in range(2):
                    nc.tensor.matmul(
                        pq[:, mj * 512:(mj + 1) * 512],
                        csb["wq8"][:, :, mj * 128:(mj + 1) * 128],
                        featsT[:, :, ch * 512:(ch + 1) * 512],
                        start=True, stop=True, perf_mode=DR)
                for mj in range(2):
                    nc.scalar.activation(
                        qm[:, ch * tpch:(ch + 1) * tpch, mj, :],
                        pq[:, mj * 512:(mj + 1) * 512]
                            .rearrange("p (t n) -> p t n", n=NPT),
                        AF.Relu,
                        bias=csb["tqv"][:, mj:mj + 1],
                        scale=csb["sqv"][:, mj:mj + 1])

            # ---- main tile loop (prod/outp run one tile behind) ------------
            prev = None  # (pv, pe, pt0) of previous tile

            def emit_prod(pv, ebb, ti):
                # prod = val'(PSUM) * ebb(SBUF)  on DVE
                prod = wpool.tile([128, 64, S], BF16, tag="prod")
                nc.vector.scalar_tensor_tensor(
                    prod[:], pv[:].rearrange("p (m s) -> p m s", s=S),
                    0.0, ebb[:].rearrange("p (m s) -> p m s", s=S),
                    op0=AO.add, op1=AO.mult)
                # S-window reduce: pool halving adds, then tiny DVE reduce
                p8 = wpool.tile([128, 64, S // 2], F32, tag="p8")
                nc.gpsimd.tensor_add(p8[:], prod[:, :, 0:8], prod[:, :, 8:16])
                p4 = wpool.tile([128, 64, S // 4], F32, tag="p4")
                nc.gpsimd.tensor_add(p4[:], p8[:, :, 0:4], p8[:, :, 4:8])
                p2 = wpool.tile([128, 64, S // 8], F32, tag="p2")
                nc.gpsimd.tensor_add(p2[:], p4[:, :, 0:2], p4[:, :, 2:4])
                nc.vector.reduce_sum(
                    outacc[:, ti, :, :].rearrange("p j n -> p (j n)"),
                    p2[:], axis=AX.X)

            for t in range(NTILES):
                pt0 = t * NPT
                fg = fgc[t // TPC]
                r0 = (t % TPC) * RT
                g0 = t * RT  # global row offset for pos

                # pos-path hidden layers; ph_mj = [h1m_mj | h1b_mj]
                h18 = wpool.tile([128, 2, 2, 512], FP8, tag="h18")
                for mj in range(2):
                    ph = psP.tile([128, 1024], F32, tag="rot")
                    for mlp, wkey in ((0, "m"), (1, "b")):
                        if h1_fp8:
                            nc.tensor.matmul(
                                ph[:, mlp * 512:(mlp + 1) * 512],
                                csb[f"w1{wkey}8"][:, :, mj * 128:(mj + 1) * 128],
                                posT[:, :, g0:g0 + RT],
                                start=True, stop=True, perf_mode=DR)
                        else:
                            nc.tensor.matmul(
                                ph[:, mlp * 512:(mlp + 1) * 512],
                                csb[f"w1{wkey}"][:, mj * 128:(mj + 1) * 128],
                                posT[:, g0:g0 + RT],
                                start=True, stop=True)
                    nc.scalar.activation(
                        h18[:, :, mj, :], ph[:].rearrange("p (l n) -> p l n", l=2),
                        AF.Relu, scale=1.0 / S_W1 if h1_fp8 else 1.0)
                h1m8 = h18[:, 0, :, :]
                h1b8 = h18[:, 1, :, :]

                # kg' = S_KW*(wk_eff @ fg + tk); t1 = relu(kg') - qm
                pk = psP.tile([128, 1024], F32, tag="rot")
                for mj in range(2):
                    nc.tensor.matmul(
                        pk[:, mj * 512:(mj + 1) * 512],
                        csb["wke8"][:, :, mj * 128:(mj + 1) * 128],
                        fg[:, :, r0:r0 + RT],
                        start=True, stop=True, perf_mode=DR)
                kgt = wpool.tile([128, 2, RT], BF16, tag="kgt")
                for mj in range(2):
                    nc.scalar.activation(
                        kgt[:, mj, :], pk[:, mj * 512:(mj + 1) * 512],
                        AF.Relu, bias=csb["tkv"][:, mj:mj + 1])
                t1 = wpool.tile([128, 64, S], BF16, tag="t1")
                qb = qm[:, t, :, :].rearrange("p j n -> p (j n)") \
                    .unsqueeze(2).broadcast_to((128, 64, S))
                nc.vector.scalar_tensor_tensor(
                    t1[:], kgt[:].rearrange("p j (n s) -> p (j n) s", s=S),
                    0.0, qb, op0=AO.add, op1=AO.subtract)

                # pem' = S_W2M*(pem + b2m); t2 = pem' * t1 (carries 32x)
                pp = psP.tile([128, 1024], F32, tag="rot")
                for mj in range(2):
                    nc.tensor.matmul(
                        pp[:, mj * 512:(mj + 1) * 512],
                        csb["w2m8"][:, :, mj * 128:(mj + 1) * 128],
                        h1m8,
                        start=True, stop=True, perf_mode=DR)
                pemb = wpool.tile([128, 2, RT], BF16, tag="pemb")
                for mj in range(2):
                    nc.scalar.activation(
                        pemb[:, mj, :], pp[:, mj * 512:(mj + 1) * 512],
                        AF.Identity, bias=csb["b2v"][:, mj:mj + 1])
                t28 = wpool.tile([128, 2, 512], FP8, tag="t28")
                nc.vector.scalar_tensor_tensor(
                    t28[:], pemb[:],
                    0.0, t1[:].rearrange("p m s -> p (m s)")
                        .rearrange("p (j n) -> p j n", j=2),
                    op0=AO.add, op1=AO.mult)

                # val' = S_VAL*(vg + peb0): both accumulated in PSUM
                pv = psP.tile([128, 1024], F32, tag="pv", bufs=1)
                for mj in range(2):
                    nc.tensor.matmul(
                        pv[:, mj * 512:(mj + 1) * 512],
                        csb["wv8"][:, :, mj * 128:(mj + 1) * 128],
                        fg[:, :, r0:r0 + RT],
                        start=True, stop=False, perf_mode=DR)
                    nc.tensor.matmul(
                        pv[:, mj * 512:(mj + 1) * 512],
                        csb["w2b8"][:, :, mj * 128:(mj + 1) * 128],
                        h1b8,
                        start=False, stop=False, perf_mode=DR)
                    nc.tensor.matmul(
                        pv[:, mj * 512:(mj + 1) * 512],
                        csb["dw2b8"][:, :, mj * 128:(mj + 1) * 128],
                        h1b8,
                        start=False, stop=True, perf_mode=DR)

                # logits: lg' = 256*lg = we18^T t28 + fw8^T h1b8
                pl = psP.tile([2 * G, RT], F32, tag="rot")
                nc.tensor.matmul(pl[:], csb["we18"][:], t28[:],
                                 start=True, stop=False, perf_mode=DR)
                nc.tensor.matmul(pl[:], csb["fw8"][:], h1b8,
                                 start=False, stop=True, perf_mode=DR)
                hw = spool.tile([G, RT], BF16, tag="hw")
                nc.scalar.activation(hw[:], pl[0:G, :], AF.Relu,
                                     bias=csb["twe"][:], scale=csb["swe"][:])
                pl2 = psP.tile([G, RT], F32, tag="rot")
                nc.tensor.matmul(pl2[:], csb["we2"][:], hw[:],
                                 start=True, stop=True)
                e = spool.tile([G, RT], BF16, tag="e")
                nc.scalar.activation(e[:], pl2[:], AF.Exp,
                                     bias=csb["web2"][:], scale=1.0)
                e8 = spool.tile([G, NPT, S // 2], F32, tag="e8")
                ev = e[:].rearrange("p (n s) -> p n s", s=S)
                nc.gpsimd.tensor_add(e8[:], ev[:, :, 0:8], ev[:, :, 8:16])
                e4 = spool.tile([G, NPT, S // 4], F32, tag="e4")
                nc.gpsimd.tensor_add(e4[:], e8[:, :, 0:4], e8[:, :, 4:8])
                nc.vector.reduce_sum(
                    esumacc[:, pt0:pt0 + NPT], e4[:], axis=AX.X)

                # expand e over channel groups; evacuate to SBUF on scalar
                pe = psP.tile([128, 1024], F32, tag="rot")
                for mj in range(2):
                    nc.tensor.matmul(
                        pe[:, mj * 512:(mj + 1) * 512],
                        csb["eoh"][:, mj, :], e[:],
                        start=True, stop=True)
                ebb = wpool.tile([128, 1024], BF16, tag="ebb")
                nc.vector.tensor_scalar_mul(ebb[:], pe[:], 1.0)

                # previous tile's prod/outp (keeps DVE queue stall-free)
                if prev is not None:
                    emit_prod(*prev)
                prev = (pv, ebb, t)

            emit_prod(*prev)

            nc.sync.dma_start(out_d[:], outacc[:])
            nc.sync.dma_start(esum_d[:], esumacc[:])

    nc.compile()
    return nc


def host_prep(inputs, h1_fp8=True):
    """Fold BN, scale/cast weights to fp8, build per-core input maps."""
    f = {k: np.asarray(v) for k, v in inputs.items()}
    feats, coords, index = f["feats"], f["coords"], f["index"]
    index = index.astype(np.int64)

    s_q, t_q = _affine(f["bnq"], f["bq"])
    s_k, t_k = _affine(f["bnk"], f["bk"])
    s_hm, t_hm = _affine(f["pm_bn"], f["pm_b1"])
    s_hb, t_hb = _affine(f["pb_bn"], f["pb_b1"])

    b2b_we = np.asarray(f["pb_b2"], np.float32) @ np.asarray(f["we_w1"], np.float32)
    s_we, t_we = _affine(f["we_bn"], np.asarray(f["we_b1"], np.float32) + b2b_we)

    wk_eff = np.asarray(f["wk"], np.float32) * s_k[None, :]
    F_mat = np.asarray(f["pb_w2"], np.float32) @ np.asarray(f["we_w1"], np.float32)

    # w1' = w1 * bn_scale with ones-row bias fold (pos row 3 == 1)
    def w1_fold(w1, s_h, t_h):
        w = np.asarray(w1, np.float32) * s_h[None, :]
        return np.concatenate([w, t_h[None, :]], 0)  # [4, C]

    w1m_f = w1_fold(f["pm_w1"], s_hm, t_hm)
    w1b_f = w1_fold(f["pb_w1"], s_hb, t_hb)

    eoh = np.zeros((G, 2, 128), np.float32)
    for g in range(G):
        j, p0 = divmod(g * 32, 128)
        eoh[g, j, p0:p0 + 32] = 1.0

    shared = {
        "wq8": (_as_lhst(f["wq"]) * S_QW).astype(NP_FP8),
        "wke8": (_as_lhst(wk_eff) * S_KW).astype(NP_FP8),
        "wv8": (_as_lhst(f["wv"]) * S_VAL).astype(NP_FP8),
        "w2m8": (_as_lhst(f["pm_w2"]) * S_W2M).astype(NP_FP8),
        "w2b8": (_as_lhst(f["pb_w2"]) * S_VAL).astype(NP_FP8),
        "dw2b8": (_as_lhst(f["pb_w2"]) * S_VAL
                  - (_as_lhst(f["pb_w2"]) * S_VAL).astype(NP_FP8)
                  .astype(np.float32)).astype(NP_FP8),
        "we18": np.concatenate(
            [(_as_lhst(f["we_w1"]) * S_WE1), np.zeros((128, 2, G), np.float32)],
            axis=2).astype(NP_FP8),
        "fw8": np.concatenate(
            [(_as_lhst(F_mat) * S_FW), np.zeros((128, 2, G), np.float32)],
            axis=2).astype(NP_FP8),
        "sqv": _per_part(s_q * S_KW / S_QW),
        "tqv": _per_part(t_q * S_KW),
        "tkv": _per_part(t_k * S_KW),
        "b2v": _per_part(np.asarray(f["pm_b2"], np.float32) * S_W2M),
        "swe": (s_we / (S_WE1 * S_KW * S_W2M)).reshape(G, 1).astype(np.float32),
        "twe": t_we.reshape(G, 1).astype(np.float32),
        "we2": np.asarray(f["we_w2"], np.float32).astype(NP_BF16),
        "web2": np.asarray(f["we_b2"], np.float32).reshape(G, 1),
        "eoh": eoh.astype(NP_BF16),
    }
    if h1_fp8:
        # k = i*2 + p mapping for [2, 2, C] lhsT / [2, 2, cols] rhs
        def pack22(w4):  # [4, C] -> [2, 2, C]
            return np.ascontiguousarray(
                w4.reshape(2, 2, -1).transpose(1, 0, 2))
        shared["w1m8"] = (pack22(w1m_f) * S_W1).astype(NP_FP8)
        shared["w1b8"] = (pack22(w1b_f) * S_W1).astype(NP_FP8)
    else:
        shared["w1m"] = w1m_f.astype(NP_BF16)
        shared["w1b"] = w1b_f.astype(NP_BF16)

    in_maps = []
    for core in range(NCORES):
        b, qc = divmod(core, CPB)
        qoff = qc * NLOC
        fb32 = np.asarray(feats[b], np.float32)
        # featsT8: [128, 2, NLOC], [p, j, n] = feats[n, j*128+p]
        fq = fb32[qoff:qoff + NLOC]
        featsT = np.ascontiguousarray(
            fq.T.reshape(2, 128, NLOC).transpose(1, 0, 2)).astype(NP_FP8)
        idx = index[b, qoff:qoff + NLOC, :].reshape(-1)
        fg = fb32[idx]                                   # [NLOC*S, C]
        fgb8 = np.ascontiguousarray(
            fg.T.reshape(2, 128, NLOC * S).transpose(1, 0, 2)).astype(NP_FP8)
        cb = np.asarray(coords[b], np.float32)
        pos = cb[qoff:qoff + NLOC][:, None, :] - cb[idx.reshape(NLOC, S)]
        pos4 = np.concatenate(
            [pos.reshape(NLOC * S, 3).T,
             np.ones((1, NLOC * S), np.float32)], 0)     # [4, NLOC*S]
        m = dict(shared)
        m["featsT8"] = featsT
        m["fgb8"] = fgb8
        if h1_fp8:
            m["pos8"] = np.ascontiguousarray(
                pos4.reshape(2, 2, NLOC * S).transpose(1, 0, 2)).astype(NP_FP8)
        else:
            m["pos4"] = pos4.astype(NP_BF16)
        in_maps.append(m)

    # host-side unshard constants
    bias = (np.asarray(f["pb_b2"], np.float32)
            + np.asarray(f["bv"], np.float32))           # [C]
    return in_maps, bias


_NC_CACHE = {}


def _h1_fp8():
    return os.environ.get("KERNEL_H1BF16", "1") != "1"


def _get_program():
    key = "nc" + ("8" if _h1_fp8() else "16")
    if key not in _NC_CACHE:
        _NC_CACHE[key] = build_program(h1_fp8=_h1_fp8())
    return _NC_CACHE[key]


def unshard(results, bias):
    out = np.zeros((B, N, C), np.float32)
    for core in range(NCORES):
        b, qc = divmod(core, CPB)
        o = np.asarray(results[core]["out"], np.float32)    # [128, T, 2, n]
        es = np.asarray(results[core]["esum"], np.float32)  # [G, NLOC]
        # channel c = j*128 + p ; group g = c // 32 ; point = t*NPT + n
        oc = o.transpose(1, 3, 2, 0).reshape(NLOC, C)       # [n, c]
        denom = np.repeat(es.T, C // G, axis=1) * S_VAL     # [n, c]
        out[b, qc * NLOC:(qc + 1) * NLOC] = oc / denom + bias[None, :]
    return out


def kernel(**inputs):
    nc = _get_program()
    in_maps, bias = host_prep(inputs, h1_fp8=_h1_fp8())
    res = bass_utils.run_bass_kernel_spmd(
        nc, in_maps, list(range(NCORES)),
        trace=bool(int(os.environ.get("KERNEL_TRACE", "0"))))
    _NC_CACHE["last_results"] = res
    return unshard(res.results, bias)


# revision 23
# speedup vs baseline: 2.1493x; 1.3326x over previous
"""Trainium2 Bass kernel for GroupedVectorSA (gnn message passing), v2.

Self-contained: accepts FULL inputs (as produced by setup_inputs()), shards
across 8 NeuronCores (batch b = core//4, quarter of N = core%4), runs one
SPMD Bass/Tile program via bass_utils.run_bass_kernel_spmd, reassembles the
full [B, N, C] output.

v2 design notes (vs v1 baseline ~700us):
  - All K=256 projections run as fp8e4(e4m3) DoubleRow matmuls (0.5 cyc/row).
  - Host pre-transposes every operand (no on-device DMA transposes) and
    pre-gathers neighbor feature rows (input-only work).
  - Linear biases enter PSUM via K=1 ones-row matmuls; BN affines fold into
    weights host-side; per-matrix power-of-2 scales keep fp8 operands in
    range and are exactly compensated downstream.
  - relu(kg')-q fused into one DVE scalar_tensor_tensor from PSUM.
  - val = vg + peb accumulated inside PSUM by the PE (no vector add).
  - softmax: unnormalized e drives the weighted sum; the denominator
    (esum) ships to HBM and the host divides during the unshard, along
    with the constant output bias (pb_b2 + bv).
  - PSUM choreography: one rotating 3-buffer [128,1024] tag for all
    short-lived psum tensors + 1 buffer for val (lives across the softmax);
    prod/outp run software-pipelined one tile behind so the in-order DVE
    queue (t1, t2, prod) never stalls.
  - Engine balance: Scalar h1 acts + hw/exp; DVE t1/t2/prod; Pool esum +
    S-window output reduce.
"""

import os
import sys

import numpy as np

try:
    import concourse  # noqa: F401
except ImportError:
    sys.path.insert(0, "/opt/trn_rl_repo")

import ml_dtypes

import concourse.bacc as bacc
import concourse.bass as bass  # noqa: F401
import concourse.mybir as mybir
import concourse.tile as tile
from concourse import bass_utils

F32 = mybir.dt.float32
BF16 = mybir.dt.bfloat16
FP16 = mybir.dt.float16
FP8 = mybir.dt.float8e4

NP_BF16 = ml_dtypes.bfloat16
NP_FP8 = ml_dtypes.float8_e4m3fn

EPS = 1e-5
B, N, S, C, G = 2, 4096, 16, 256, 8
NCORES = 8
CPB = NCORES // B          # cores per batch = 4
NLOC = N // CPB            # points per core = 1024
NPT = 32                   # points per compute tile
RT = NPT * S               # gathered rows per compute tile = 512
NTILES = NLOC // NPT       # 32
NCHUNK = 4                 # fgT8 DMA chunks
TPC = NTILES // NCHUNK     # tiles per chunk = 8
CCOLS = NLOC * S // NCHUNK  # columns per chunk = 4096

# power-of-2 fp8 range scales (exactly compensated downstream)
S_QW = 8.0     # wq
S_KW = 4.0     # wk_eff; kg-psum, t1, qm carry S_KW
S_W2M = 2.0    # pm_w2; pem-psum carries S_W2M -> t2 carries S_KW*S_W2M=8
S_WE1 = 16.0   # we_w1; lg carries S_WE1*8 = 128
S_FW = 128.0   # F = pb_w2 @ we_w1 (must equal S_WE1*S_KW*S_W2M)
S_VAL = 8.0    # wv and pb_w2 (val-psum, outacc carry S_VAL)
S_W1 = 4.0     # pm_w1/pb_w1 fp8 lhsT scale (h1 act divides back)

AO = mybir.AluOpType
AF = mybir.ActivationFunctionType
AX = mybir.AxisListType
DR = mybir.MatmulPerfMode.DoubleRow


def _affine(bn_p, lin_b):
    """Fold eval-mode BN (+ preceding linear bias) into scale/bias vectors."""
    bn_p = np.asarray(bn_p, np.float32)
    g, beta, m, v = bn_p[0], bn_p[1], bn_p[2], bn_p[3]
    s = g / np.sqrt(v + EPS)
    t = (np.asarray(lin_b, np.float32) - m) * s + beta
    return s.astype(np.float32), t.astype(np.float32)


def _as_lhst(w):
    """[256, X] -> [128, 2, X] (partition p, k-half j: k = j*128 + p)."""
    w = np.asarray(w, np.float32)
    return np.ascontiguousarray(w.reshape(2, 128, w.shape[1]).transpose(1, 0, 2))


def _per_part(vec):
    """[256] -> [128, 2]  (channel = j*128 + p)."""
    return np.ascontiguousarray(np.asarray(vec, np.float32).reshape(2, 128).T)


def build_program(h1_fp8=True):
    nc = bacc.Bacc("TRN2", target_bir_lowering=False, debug=False,
                   num_devices=NCORES)

    def din(name, shape, dt):
        return nc.dram_tensor(name, list(shape), dt, kind="ExternalInput")

    featsT8 = din("featsT8", [128, 2, NLOC], FP8)
    fgb8 = din("fgb8", [128, 2, NLOC * S], FP8)
    if h1_fp8:
        pos_d = din("pos8", [2, 2, NLOC * S], FP8)
    else:
        pos_d = din("pos4", [4, NLOC * S], BF16)
    consts = [
        ("wq8", [128, 2, C], FP8), ("wke8", [128, 2, C], FP8),
        ("wv8", [128, 2, C], FP8),
        ("w2m8", [128, 2, C], FP8), ("w2b8", [128, 2, C], FP8),
        ("dw2b8", [128, 2, C], FP8),
        ("we18", [128, 2, 2 * G], FP8), ("fw8", [128, 2, 2 * G], FP8),
        ("sqv", [128, 2], F32), ("tqv", [128, 2], F32),
        ("tkv", [128, 2], F32), ("b2v", [128, 2], F32),
        ("swe", [G, 1], F32), ("twe", [G, 1], F32),
        ("we2", [G, G], BF16), ("web2", [G, 1], F32),
        ("eoh", [G, 2, 128], BF16),
    ]
    if h1_fp8:
        consts += [("w1m8", [2, 2, C], FP8), ("w1b8", [2, 2, C], FP8)]
    else:
        consts += [("w1m", [4, C], BF16), ("w1b", [4, C], BF16)]
    cdram = {name: din(name, shape, dt) for name, shape, dt in consts}

    out_d = nc.dram_tensor("out", [128, NTILES, 2, NPT], F32,
                           kind="ExternalOutput")
    esum_d = nc.dram_tensor("esum", [G, NLOC], F32, kind="ExternalOutput")

    with tile.TileContext(nc) as tc:
        with (
            tc.tile_pool(name="const", bufs=1) as cpool,
            tc.tile_pool(name="big", bufs=1) as bigpool,
            tc.tile_pool(name="work", bufs=2) as wpool,
            tc.tile_pool(name="small", bufs=3) as spool,
            tc.tile_pool(name="ps", bufs=2, space="PSUM") as psP,
            tc.tile_pool(name="psm", bufs=1, space="PSUM") as psM,
        ):
            csb = {}
            for name, shape, dt in consts:
                t = cpool.tile(list(shape), dt, tag=name)
                nc.sync.dma_start(t[:], cdram[name][:])
                csb[name] = t

            featsT = cpool.tile([128, 2, NLOC], FP8, tag="featsT")
            nc.sync.dma_start(featsT[:], featsT8[:])
            posT = cpool.tile(
                [2, 2, NLOC * S] if h1_fp8 else [4, NLOC * S],
                FP8 if h1_fp8 else BF16, tag="posT")
            nc.sync.dma_start(posT[:], pos_d[:])
            fgc = []
            for cch in range(NCHUNK):
                t = cpool.tile([128, 2, CCOLS], FP8, tag=f"fg{cch}")
                nc.sync.dma_start(t[:], fgb8[:, :, cch * CCOLS:(cch + 1) * CCOLS])
                fgc.append(t)

            # tile-local layout [p, tile, j, n] so (j, n) flattens contiguous
            outacc = bigpool.tile([128, NTILES, 2, NPT], F32, tag="outacc")
            esumacc = bigpool.tile([G, NLOC], F32, tag="esumacc")

            # ---- q phase: qm = S_KW * relu(bn_q(feats @ wq + bq)) ----------
            # tile-local layout [p, tile, j, n] so (j, n) flattens contiguous
            qm = bigpool.tile([128, NTILES, 2, NPT], BF16, tag="qm")
            tpch = 512 // NPT  # tiles per 512-point chunk
            for ch in range(NLOC // 512):
                pq = psP.tile([128, 1024], F32, tag="rot")
                for mj in range(2):
                    nc.tensor.matmul(
                        pq[:, mj * 512:(mj + 1) * 512],
                        csb["wq8"][:, :, mj * 128:(mj + 1) * 128],
                        featsT[:, :, ch * 512:(ch + 1) * 512],
                        start=True, stop=True, perf_mode=DR)
                for mj in range(2):
                    nc.scalar.activation(
                        qm[:, ch * tpch:(ch + 1) * tpch, mj, :],
                        pq[:, mj * 512:(mj + 1) * 512]
                            .rearrange("p (t n) -> p t n", n=NPT),
                        AF.Relu,
                        bias=csb["tqv"][:, mj:mj + 1],
                        scale=csb["sqv"][:, mj:mj + 1])

            # ---- main tile loop (prod/outp run one tile behind) ------------
            prev = None  # (pv, pe, pt0) of previous tile

            def emit_prod(pv, ebb, ti):
                # prod = val'(PSUM) * ebb(SBUF)  on DVE
                prod = wpool.tile([128, 64, S], BF16, tag="prod")
                nc.vector.scalar_tensor_tensor(
                    prod[:], pv[:].rearrange("p (m s) -> p m s", s=S),
                    0.0, ebb[:].rearrange("p (m s) -> p m s", s=S),
                    op0=AO.add, op1=AO.mult)
                # S-window reduce: pool halving adds, then tiny DVE reduce
                p8 = wpool.tile([128, 64, S // 2], F32, tag="p8")
                nc.gpsimd.tensor_add(p8[:], prod[:, :, 0:8], prod[:, :, 8:16])
                p4 = wpool.tile([128, 64, S // 4], F32, tag="p4")
                nc.gpsimd.tensor_add(p4[:], p8[:, :, 0:4], p8[:, :, 4:8])
                p2 = wpool.tile([128, 64, S // 8], F32, tag="p2")
                nc.gpsimd.tensor_add(p2[:], p4[:, :, 0:2], p4[:, :, 2:4])
                nc.vector.reduce_sum(
                    outacc[:, ti, :, :].rearrange("p j n -> p (j n)"),
                    p2[:], axis=AX.X)

            for t in range(NTILES):
                pt0 = t * NPT
                fg = fgc[t // TPC]
                r0 = (t % TPC) * RT
                g0 = t * RT  # global row offset for pos

                # pos-path hidden layers; ph_mj = [h1m_mj | h1b_mj]
                h18 = wpool.tile([128, 2, 2, 512], FP8, tag="h18")
                for mj in range(2):
                    ph = psP.tile([128, 1024], F32, tag="rot")
                    for mlp, wkey in ((0, "m"), (1, "b")):
                        if h1_fp8:
                            nc.tensor.matmul(
                                ph[:, mlp * 512:(mlp + 1) * 512],
                                csb[f"w1{wkey}8"][:, :, mj * 128:(mj + 1) * 128],
                                posT[:, :, g0:g0 + RT],
                                start=True, stop=True, perf_mode=DR)
                        else:
                            nc.tensor.matmul(
                                ph[:, mlp * 512:(mlp + 1) * 512],
                                csb[f"w1{wkey}"][:, mj * 128:(mj + 1) * 128],
                                posT[:, g0:g0 + RT],
                                start=True, stop=True)
                    nc.scalar.activation(
                        h18[:, :, mj, :], ph[:].rearrange("p (l n) -> p l n", l=2),
                        AF.Relu, scale=1.0 / S_W1 if h1_fp8 else 1.0)
                h1m8 = h18[:, 0, :, :]
                h1b8 = h18[:, 1, :, :]

                # kg' = S_KW*(wk_eff @ fg + tk); t1 = relu(kg') - qm
                pk = psP.tile([128, 1024], F32, tag="rot")
                for mj in range(2):
                    nc.tensor.matmul(
                        pk[:, mj * 512:(mj + 1) * 512],
                        csb["wke8"][:, :, mj * 128:(mj + 1) * 128],
                        fg[:, :, r0:r0 + RT],
                        start=True, stop=True, perf_mode=DR)
                kgt = wpool.tile([128, 2, RT], BF16, tag="kgt")
                for mj in range(2):
                    nc.scalar.activation(
                        kgt[:, mj, :], pk[:, mj * 512:(mj + 1) * 512],
                        AF.Relu, bias=csb["tkv"][:, mj:mj + 1])
                t1 = wpool.tile([128, 64, S], BF16, tag="t1")
                qb = qm[:, t, :, :].rearrange("p j n -> p (j n)") \
                    .unsqueeze(2).broadcast_to((128, 64, S))
                nc.vector.scalar_tensor_tensor(
                    t1[:], kgt[:].rearrange("p j (n s) -> p (j n) s", s=S),
                    0.0, qb, op0=AO.add, op1=AO.subtract)

                # pem' = S_W2M*(pem + b2m); t2 = pem' * t1 (carries 32x)
                pp = psP.tile([128, 1024], F32, tag="rot")
                for mj in range(2):
                    nc.tensor.matmul(
                        pp[:, mj * 512:(mj + 1) * 512],
                        csb["w2m8"][:, :, mj * 128:(mj + 1) * 128],
                        h1m8,
                        start=True, stop=True, perf_mode=DR)
                pemb = wpool.tile([128, 2, RT], BF16, tag="pemb")
                for mj in range(2):
                    nc.scalar.activation(
                        pemb[:, mj, :], pp[:, mj * 512:(mj + 1) * 512],
                        AF.Identity, bias=csb["b2v"][:, mj:mj + 1])
                t28 = wpool.tile([128, 2, 512], FP8, tag="t28")
                nc.vector.scalar_tensor_tensor(
                    t28[:], pemb[:],
                    0.0, t1[:].rearrange("p m s -> p (m s)")
                        .rearrange("p (j n) -> p j n", j=2),
                    op0=AO.add, op1=AO.mult)

                # val' = S_VAL*(vg + peb0): both accumulated in PSUM
                pv = psP.tile([128, 1024], F32, tag="pv", bufs=1)
                for mj in range(2):
                    nc.tensor.matmul(
                        pv[:, mj * 512:(mj + 1) * 512],
                        csb["wv8"][:, :, mj * 128:(mj + 1) * 128],
                        fg[:, :, r0:r0 + RT],
                        start=True, stop=False, perf_mode=DR)
                    nc.tensor.matmul(
                        pv[:, mj * 512:(mj + 1) * 512],
                        csb["w2b8"][:, :, mj * 128:(mj + 1) * 128],
                        h1b8,
                        start=False, stop=False, perf_mode=DR)
                    nc.tensor.matmul(
                        pv[:, mj * 512:(mj + 1) * 512],
                        csb["dw2b8"][:, :, mj * 128:(mj + 1) * 128],
                        h1b8,
                        start=False, stop=True, perf_mode=DR)

                # logits: lg' = 256*lg = we18^T t28 + fw8^T h1b8
                pl = psM.tile([2 * G, RT], F32, tag="misc",
                              padded_shape=[128, 1024])
                nc.tensor.matmul(pl[:], csb["we18"][:], t28[:],
                                 start=True, stop=False, perf_mode=DR)
                nc.tensor.matmul(pl[:], csb["fw8"][:], h1b8,
                                 start=False, stop=True, perf_mode=DR)
                hw = spool.tile([G, RT], BF16, tag="hw")
                nc.scalar.activation(hw[:], pl[0:G, :], AF.Relu,
                                     bias=csb["twe"][:], scale=csb["swe"][:])
                pl2 = psM.tile([G, RT], F32, tag="misc",
                               padded_shape=[128, 1024])
                nc.tensor.matmul(pl2[:], csb["we2"][:], hw[:],
                                 start=True, stop=True)
                e = spool.tile([G, RT], BF16, tag="e")
                nc.scalar.activation(e[:], pl2[:], AF.Exp,
                                     bias=csb["web2"][:], scale=1.0)
                e8 = spool.tile([G, NPT, S // 2], F32, tag="e8")
                ev = e[:].rearrange("p (n s) -> p n s", s=S)
                nc.gpsimd.tensor_add(e8[:], ev[:, :, 0:8], ev[:, :, 8:16])
                e4 = spool.tile([G, NPT, S // 4], F32, tag="e4")
                nc.gpsimd.tensor_add(e4[:], e8[:, :, 0:4], e8[:, :, 4:8])
                nc.vector.reduce_sum(
                    esumacc[:, pt0:pt0 + NPT], e4[:], axis=AX.X)

                # expand e over channel groups; evacuate to SBUF on scalar
                pe = psM.tile([128, 1024], F32, tag="misc")
                for mj in range(2):
                    nc.tensor.matmul(
                        pe[:, mj * 512:(mj + 1) * 512],
                        csb["eoh"][:, mj, :], e[:],
                        start=True, stop=True)
                ebb = wpool.tile([128, 1024], BF16, tag="ebb")
                nc.vector.tensor_scalar_mul(ebb[:], pe[:], 1.0)

                # previous tile's prod/outp (keeps DVE queue stall-free)
                if prev is not None:
                    emit_prod(*prev)
                prev = (pv, ebb, t)

            emit_prod(*prev)

            nc.sync.dma_start(out_d[:], outacc[:])
            nc.sync.dma_start(esum_d[:], esumacc[:])

    nc.compile()
    return nc


def host_prep(inputs, h1_fp8=True):
    """Fold BN, scale/cast weights to fp8, build per-core input maps."""
    f = {k: np.asarray(v) for k, v in inputs.items()}
    feats, coords, index = f["feats"], f["coords"], f["index"]
    index = index.astype(np.int64)

    s_q, t_q = _affine(f["bnq"], f["bq"])
    s_k, t_k = _affine(f["bnk"], f["bk"])
    s_hm, t_hm = _affine(f["pm_bn"], f["pm_b1"])
    s_hb, t_hb = _affine(f["pb_bn"], f["pb_b1"])

    b2b_we = np.asarray(f["pb_b2"], np.float32) @ np.asarray(f["we_w1"], np.float32)
    s_we, t_we = _affine(f["we_bn"], np.asarray(f["we_b1"], np.float32) + b2b_we)

    wk_eff = np.asarray(f["wk"], np.float32) * s_k[None, :]
    F_mat = np.asarray(f["pb_w2"], np.float32) @ np.asarray(f["we_w1"], np.float32)

    # w1' = w1 * bn_scale with ones-row bias fold (pos row 3 == 1)
    def w1_fold(w1, s_h, t_h):
        w = np.asarray(w1, np.float32) * s_h[None, :]
        return np.concatenate([w, t_h[None, :]], 0)  # [4, C]

    w1m_f = w1_fold(f["pm_w1"], s_hm, t_hm)
    w1b_f = w1_fold(f["pb_w1"], s_hb, t_hb)

    eoh = np.zeros((G, 2, 128), np.float32)
    for g in range(G):
        j, p0 = divmod(g * 32, 128)
        eoh[g, j, p0:p0 + 32] = 1.0

    shared = {
        "wq8": (_as_lhst(f["wq"]) * S_QW).astype(NP_FP8),
        "wke8": (_as_lhst(wk_eff) * S_KW).astype(NP_FP8),
        "wv8": (_as_lhst(f["wv"]) * S_VAL).astype(NP_FP8),
        "w2m8": (_as_lhst(f["pm_w2"]) * S_W2M).astype(NP_FP8),
        "w2b8": (_as_lhst(f["pb_w2"]) * S_VAL).astype(NP_FP8),
        "dw2b8": (_as_lhst(f["pb_w2"]) * S_VAL
                  - (_as_lhst(f["pb_w2"]) * S_VAL).astype(NP_FP8)
                  .astype(np.float32)).astype(NP_FP8),
        "we18": np.concatenate(
            [(_as_lhst(f["we_w1"]) * S_WE1), np.zeros((128, 2, G), np.float32)],
            axis=2).astype(NP_FP8),
        "fw8": np.concatenate(
            [(_as_lhst(F_mat) * S_FW), np.zeros((128, 2, G), np.float32)],
            axis=2).astype(NP_FP8),
        "sqv": _per_part(s_q * S_KW / S_QW),
        "tqv": _per_part(t_q * S_KW),
        "tkv": _per_part(t_k * S_KW),
        "b2v": _per_part(np.asarray(f["pm_b2"], np.float32) * S_W2M),
        "swe": (s_we / (S_WE1 * S_KW * S_W2M)).reshape(G, 1).astype(np.float32),
        "twe": t_we.reshape(G, 1).astype(np.float32),
        "we2": np.asarray(f["we_w2"], np.float32).astype(NP_BF16),
        "web2": np.asarray(f["we_b2"], np.float32).reshape(G, 1),
        "eoh": eoh.astype(NP_BF16),
    }
    if h1_fp8:
        # k = i*2 + p mapping for [2, 2, C] lhsT / [2, 2, cols] rhs
        def pack22(w4):  # [4, C] -> [2, 2, C]
            return np.ascontiguousarray(
                w4.reshape(2, 2, -1).transpose(1, 0, 2))
        shared["w1m8"] = (pack22(w1m_f) * S_W1).astype(NP_FP8)
        shared["w1b8"] = (pack22(w1b_f) * S_W1).astype(NP_FP8)
    else:
        shared["w1m"] = w1m_f.astype(NP_BF16)
        shared["w1b"] = w1b_f.astype(NP_BF16)

    in_maps = []
    for core in range(NCORES):
        b, qc = divmod(core, CPB)
        qoff = qc * NLOC
        fb32 = np.asarray(feats[b], np.float32)
        # featsT8: [128, 2, NLOC], [p, j, n] = feats[n, j*128+p]
        fq = fb32[qoff:qoff + NLOC]
        featsT = np.ascontiguousarray(
            fq.T.reshape(2, 128, NLOC).transpose(1, 0, 2)).astype(NP_FP8)
        idx = index[b, qoff:qoff + NLOC, :].reshape(-1)
        fg = fb32[idx]                                   # [NLOC*S, C]
        fgb8 = np.ascontiguousarray(
            fg.T.reshape(2, 128, NLOC * S).transpose(1, 0, 2)).astype(NP_FP8)
        cb = np.asarray(coords[b], np.float32)
        pos = cb[qoff:qoff + NLOC][:, None, :] - cb[idx.reshape(NLOC, S)]
        pos4 = np.concatenate(
            [pos.reshape(NLOC * S, 3).T,
             np.ones((1, NLOC * S), np.float32)], 0)     # [4, NLOC*S]
        m = dict(shared)
        m["featsT8"] = featsT
        m["fgb8"] = fgb8
        if h1_fp8:
            m["pos8"] = np.ascontiguousarray(
                pos4.reshape(2, 2, NLOC * S).transpose(1, 0, 2)).astype(NP_FP8)
        else:
            m["pos4"] = pos4.astype(NP_BF16)
        in_maps.append(m)

    # host-side unshard constants
    bias = (np.asarray(f["pb_b2"], np.float32)
            + np.asarray(f["bv"], np.float32))           # [C]
    return in_maps, bias


_NC_CACHE = {}


def _h1_fp8():
    return os.environ.get("KERNEL_H1BF16", "1") != "1"


def _get_program():
    key = "nc" + ("8" if _h1_fp8() else "16")
    if key not in _NC_CACHE:
        _NC_CACHE[key] = build_program(h1_fp8=_h1_fp8())
    return _NC_CACHE[key]


def unshard(results, bias):
    out = np.zeros((B, N, C), np.float32)
    for core in range(NCORES):
        b, qc = divmod(core, CPB)
        o = np.asarray(results[core]["out"], np.float32)    # [128, T, 2, n]
        es = np.asarray(results[core]["esum"], np.float32)  # [G, NLOC]
        # channel c = j*128 + p ; group g = c // 32 ; point = t*NPT + n
        oc = o.transpose(1, 3, 2, 0).reshape(NLOC, C)       # [n, c]
        denom = np.repeat(es.T, C // G, axis=1) * S_VAL     # [n, c]
        out[b, qc * NLOC:(qc + 1) * NLOC] = oc / denom + bias[None, :]
    return out


def kernel(**inputs):
    nc = _get_program()
    in_maps, bias = host_prep(inputs, h1_fp8=_h1_fp8())
    res = bass_utils.run_bass_kernel_spmd(
        nc, in_maps, list(range(NCORES)),
        trace=bool(int(os.environ.get("KERNEL_TRACE", "0"))))
    _NC_CACHE["last_results"] = res
    return unshard(res.results, bias)


# revision 25
# speedup vs baseline: 2.1538x; 1.0021x over previous
"""Trainium2 Bass kernel for GroupedVectorSA (gnn message passing), v2.

Self-contained: accepts FULL inputs (as produced by setup_inputs()), shards
across 8 NeuronCores (batch b = core//4, quarter of N = core%4), runs one
SPMD Bass/Tile program via bass_utils.run_bass_kernel_spmd, reassembles the
full [B, N, C] output.

v2 design notes (vs v1 baseline ~700us):
  - All K=256 projections run as fp8e4(e4m3) DoubleRow matmuls (0.5 cyc/row).
  - Host pre-transposes every operand (no on-device DMA transposes) and
    pre-gathers neighbor feature rows (input-only work).
  - Linear biases enter PSUM via K=1 ones-row matmuls; BN affines fold into
    weights host-side; per-matrix power-of-2 scales keep fp8 operands in
    range and are exactly compensated downstream.
  - relu(kg')-q fused into one DVE scalar_tensor_tensor from PSUM.
  - val = vg + peb accumulated inside PSUM by the PE (no vector add).
  - softmax: unnormalized e drives the weighted sum; the denominator
    (esum) ships to HBM and the host divides during the unshard, along
    with the constant output bias (pb_b2 + bv).
  - PSUM choreography: one rotating 3-buffer [128,1024] tag for all
    short-lived psum tensors + 1 buffer for val (lives across the softmax);
    prod/outp run software-pipelined one tile behind so the in-order DVE
    queue (t1, t2, prod) never stalls.
  - Engine balance: Scalar h1 acts + hw/exp; DVE t1/t2/prod; Pool esum +
    S-window output reduce.
"""

import os
import sys

import numpy as np

try:
    import concourse  # noqa: F401
except ImportError:
    sys.path.insert(0, "/opt/trn_rl_repo")

import ml_dtypes

import concourse.bacc as bacc
import concourse.bass as bass  # noqa: F401
import concourse.mybir as mybir
import concourse.tile as tile
from concourse import bass_utils

F32 = mybir.dt.float32
BF16 = mybir.dt.bfloat16
FP16 = mybir.dt.float16
FP8 = mybir.dt.float8e4

NP_BF16 = ml_dtypes.bfloat16
NP_FP8 = ml_dtypes.float8_e4m3fn

EPS = 1e-5
B, N, S, C, G = 2, 4096, 16, 256, 8
NCORES = 8
CPB = NCORES // B          # cores per batch = 4
NLOC = N // CPB            # points per core = 1024
NPT = 32                   # points per compute tile
RT = NPT * S               # gathered rows per compute tile = 512
NTILES = NLOC // NPT       # 32
NCHUNK = 4                 # fgT8 DMA chunks
TPC = NTILES // NCHUNK     # tiles per chunk = 8
CCOLS = NLOC * S // NCHUNK  # columns per chunk = 4096

# power-of-2 fp8 range scales (exactly compensated downstream)
S_QW = 8.0     # wq
S_KW = 4.0     # wk_eff; kg-psum, t1, qm carry S_KW
S_W2M = 2.0    # pm_w2; pem-psum carries S_W2M -> t2 carries S_KW*S_W2M=8
S_WE1 = 16.0   # we_w1; lg carries S_WE1*8 = 128
S_FW = 128.0   # F = pb_w2 @ we_w1 (must equal S_WE1*S_KW*S_W2M)
S_VAL = 8.0    # wv and pb_w2 (val-psum, outacc carry S_VAL)
S_W1 = 4.0     # pm_w1/pb_w1 fp8 lhsT scale (h1 act divides back)

AO = mybir.AluOpType
AF = mybir.ActivationFunctionType
AX = mybir.AxisListType
DR = mybir.MatmulPerfMode.DoubleRow


def _affine(bn_p, lin_b):
    """Fold eval-mode BN (+ preceding linear bias) into scale/bias vectors."""
    bn_p = np.asarray(bn_p, np.float32)
    g, beta, m, v = bn_p[0], bn_p[1], bn_p[2], bn_p[3]
    s = g / np.sqrt(v + EPS)
    t = (np.asarray(lin_b, np.float32) - m) * s + beta
    return s.astype(np.float32), t.astype(np.float32)


def _as_lhst(w):
    """[256, X] -> [128, 2, X] (partition p, k-half j: k = j*128 + p)."""
    w = np.asarray(w, np.float32)
    return np.ascontiguousarray(w.reshape(2, 128, w.shape[1]).transpose(1, 0, 2))


def _per_part(vec):
    """[256] -> [128, 2]  (channel = j*128 + p)."""
    return np.ascontiguousarray(np.asarray(vec, np.float32).reshape(2, 128).T)


def build_program(h1_fp8=True):
    nc = bacc.Bacc("TRN2", target_bir_lowering=False, debug=False,
                   num_devices=NCORES)

    def din(name, shape, dt):
        return nc.dram_tensor(name, list(shape), dt, kind="ExternalInput")

    featsT8 = din("featsT8", [128, 2, NLOC], FP8)
    fgb8 = din("fgb8", [128, 2, NLOC * S], FP8)
    if h1_fp8:
        pos_d = din("pos8", [2, 2, NLOC * S], FP8)
    else:
        pos_d = din("pos4", [4, NLOC * S], BF16)
    consts = [
        ("wq8", [128, 2, C], FP8), ("wke8", [128, 2, C], FP8),
        ("wv8", [128, 2, C], FP8),
        ("w2m8", [128, 2, C], FP8), ("w2b8", [128, 2, C], FP8),
        ("dw2b8", [128, 2, C], FP8),
        ("we18", [128, 2, 2 * G], FP8), ("fw8", [128, 2, 2 * G], FP8),
        ("sqv", [128, 2], F32), ("tqv", [128, 2], F32),
        ("tkv", [128, 2], F32), ("b2v", [128, 2], F32),
        ("swe", [G, 1], F32), ("twe", [G, 1], F32),
        ("we2", [G, G], BF16), ("web2", [G, 1], F32),
        ("eoh", [G, 2, 128], BF16),
    ]
    if h1_fp8:
        consts += [("w1m8", [2, 2, C], FP8), ("w1b8", [2, 2, C], FP8)]
    else:
        consts += [("w1m", [4, C], BF16), ("w1b", [4, C], BF16)]
    cdram = {name: din(name, shape, dt) for name, shape, dt in consts}

    out_d = nc.dram_tensor("out", [128, NTILES, 2, NPT], F32,
                           kind="ExternalOutput")
    esum_d = nc.dram_tensor("esum", [G, NLOC], F32, kind="ExternalOutput")

    with tile.TileContext(nc) as tc:
        with (
            tc.tile_pool(name="const", bufs=1) as cpool,
            tc.tile_pool(name="big", bufs=1) as bigpool,
            tc.tile_pool(name="work", bufs=2) as wpool,
            tc.tile_pool(name="small", bufs=3) as spool,
            tc.tile_pool(name="ps", bufs=2, space="PSUM") as psP,
            tc.tile_pool(name="psm", bufs=1, space="PSUM") as psM,
        ):
            csb = {}
            for name, shape, dt in consts:
                t = cpool.tile(list(shape), dt, tag=name)
                nc.sync.dma_start(t[:], cdram[name][:])
                csb[name] = t

            featsT = cpool.tile([128, 2, NLOC], FP8, tag="featsT")
            nc.sync.dma_start(featsT[:], featsT8[:])
            posT = cpool.tile(
                [2, 2, NLOC * S] if h1_fp8 else [4, NLOC * S],
                FP8 if h1_fp8 else BF16, tag="posT")
            nc.sync.dma_start(posT[:], pos_d[:])
            fgc = []
            for cch in range(NCHUNK):
                t = cpool.tile([128, 2, CCOLS], FP8, tag=f"fg{cch}")
                nc.sync.dma_start(t[:], fgb8[:, :, cch * CCOLS:(cch + 1) * CCOLS])
                fgc.append(t)

            # tile-local layout [p, tile, j, n] so (j, n) flattens contiguous
            outacc = bigpool.tile([128, NTILES, 2, NPT], F32, tag="outacc")
            esumacc = bigpool.tile([G, NLOC], F32, tag="esumacc")

            # ---- q phase: qm = S_KW * relu(bn_q(feats @ wq + bq)) ----------
            # tile-local layout [p, tile, j, n] so (j, n) flattens contiguous
            qm = bigpool.tile([128, NTILES, 2, NPT], BF16, tag="qm")
            tpch = 512 // NPT  # tiles per 512-point chunk
            for ch in range(NLOC // 512):
                pq = psP.tile([128, 1024], F32, tag="rot")
                for mj in range(2):
                    nc.tensor.matmul(
                        pq[:, mj * 512:(mj + 1) * 512],
                        csb["wq8"][:, :, mj * 128:(mj + 1) * 128],
                        featsT[:, :, ch * 512:(ch + 1) * 512],
                        start=True, stop=True, perf_mode=DR)
                for mj in range(2):
                    nc.scalar.activation(
                        qm[:, ch * tpch:(ch + 1) * tpch, mj, :],
                        pq[:, mj * 512:(mj + 1) * 512]
                            .rearrange("p (t n) -> p t n", n=NPT),
                        AF.Relu,
                        bias=csb["tqv"][:, mj:mj + 1],
                        scale=csb["sqv"][:, mj:mj + 1])

            # ---- main tile loop ------------------------------------------
            # h1 stage runs one tile AHEAD; prod/outp one tile BEHIND.
            prev = None  # (pv, ebb, ti) of previous tile

            def emit_h1(t):
                g0 = t * RT
                h18 = wpool.tile([128, 2, 2, 512], FP8, tag="h18")
                for mj in range(2):
                    ph = psP.tile([128, 1024], F32, tag="rot")
                    for mlp, wkey in ((0, "m"), (1, "b")):
                        if h1_fp8:
                            nc.tensor.matmul(
                                ph[:, mlp * 512:(mlp + 1) * 512],
                                csb[f"w1{wkey}8"][:, :, mj * 128:(mj + 1) * 128],
                                posT[:, :, g0:g0 + RT],
                                start=True, stop=True, perf_mode=DR)
                        else:
                            nc.tensor.matmul(
                                ph[:, mlp * 512:(mlp + 1) * 512],
                                csb[f"w1{wkey}"][:, mj * 128:(mj + 1) * 128],
                                posT[:, g0:g0 + RT],
                                start=True, stop=True)
                    nc.scalar.activation(
                        h18[:, :, mj, :], ph[:].rearrange("p (l n) -> p l n", l=2),
                        AF.Relu, scale=1.0 / S_W1 if h1_fp8 else 1.0)
                return h18

            def emit_prod(pv, ebb, ti):
                # prod = val'(PSUM) * ebb(SBUF)  on DVE
                prod = wpool.tile([128, 64, S], BF16, tag="prod")
                nc.vector.scalar_tensor_tensor(
                    prod[:], pv[:].rearrange("p (m s) -> p m s", s=S),
                    0.0, ebb[:].rearrange("p (m s) -> p m s", s=S),
                    op0=AO.add, op1=AO.mult)
                # S-window reduce: pool halving adds, then tiny DVE reduce
                p8 = wpool.tile([128, 64, S // 2], F32, tag="p8")
                nc.gpsimd.tensor_add(p8[:], prod[:, :, 0:8], prod[:, :, 8:16])
                p4 = wpool.tile([128, 64, S // 4], F32, tag="p4")
                nc.gpsimd.tensor_add(p4[:], p8[:, :, 0:4], p8[:, :, 4:8])
                p2 = wpool.tile([128, 64, S // 8], F32, tag="p2")
                nc.gpsimd.tensor_add(p2[:], p4[:, :, 0:2], p4[:, :, 2:4])
                nc.vector.reduce_sum(
                    outacc[:, ti, :, :].rearrange("p j n -> p (j n)"),
                    p2[:], axis=AX.X)

            h18_cur = emit_h1(0)

            for t in range(NTILES):
                pt0 = t * NPT
                fg = fgc[t // TPC]
                r0 = (t % TPC) * RT

                # pos-path hidden layers were computed one tile ahead
                h18 = h18_cur
                h1m8 = h18[:, 0, :, :]
                h1b8 = h18[:, 1, :, :]

                # kg' = S_KW*(wk_eff @ fg + tk); t1 = relu(kg') - qm
                pk = psP.tile([128, 1024], F32, tag="rot")
                for mj in range(2):
                    nc.tensor.matmul(
                        pk[:, mj * 512:(mj + 1) * 512],
                        csb["wke8"][:, :, mj * 128:(mj + 1) * 128],
                        fg[:, :, r0:r0 + RT],
                        start=True, stop=True, perf_mode=DR)
                kgt = wpool.tile([128, 2, RT], BF16, tag="kgt")
                for mj in range(2):
                    nc.scalar.activation(
                        kgt[:, mj, :], pk[:, mj * 512:(mj + 1) * 512],
                        AF.Relu, bias=csb["tkv"][:, mj:mj + 1])
                t1 = wpool.tile([128, 64, S], BF16, tag="t1")
                qb = qm[:, t, :, :].rearrange("p j n -> p (j n)") \
                    .unsqueeze(2).broadcast_to((128, 64, S))
                nc.vector.scalar_tensor_tensor(
                    t1[:], kgt[:].rearrange("p j (n s) -> p (j n) s", s=S),
                    0.0, qb, op0=AO.add, op1=AO.subtract)

                # pem' = S_W2M*(pem + b2m); t2 = pem' * t1 (carries 32x)
                pp = psP.tile([128, 1024], F32, tag="rot")
                for mj in range(2):
                    nc.tensor.matmul(
                        pp[:, mj * 512:(mj + 1) * 512],
                        csb["w2m8"][:, :, mj * 128:(mj + 1) * 128],
                        h1m8,
                        start=True, stop=True, perf_mode=DR)
                pemb = wpool.tile([128, 2, RT], BF16, tag="pemb")
                for mj in range(2):
                    nc.scalar.activation(
                        pemb[:, mj, :], pp[:, mj * 512:(mj + 1) * 512],
                        AF.Identity, bias=csb["b2v"][:, mj:mj + 1])
                t28 = wpool.tile([128, 2, 512], FP8, tag="t28")
                nc.vector.scalar_tensor_tensor(
                    t28[:], pemb[:],
                    0.0, t1[:].rearrange("p m s -> p (m s)")
                        .rearrange("p (j n) -> p j n", j=2),
                    op0=AO.add, op1=AO.mult)

                # val' = S_VAL*(vg + peb0): both accumulated in PSUM
                pv = psP.tile([128, 1024], F32, tag="pv", bufs=1)
                for mj in range(2):
                    nc.tensor.matmul(
                        pv[:, mj * 512:(mj + 1) * 512],
                        csb["wv8"][:, :, mj * 128:(mj + 1) * 128],
                        fg[:, :, r0:r0 + RT],
                        start=True, stop=False, perf_mode=DR)
                    nc.tensor.matmul(
                        pv[:, mj * 512:(mj + 1) * 512],
                        csb["w2b8"][:, :, mj * 128:(mj + 1) * 128],
                        h1b8,
                        start=False, stop=False, perf_mode=DR)
                    nc.tensor.matmul(
                        pv[:, mj * 512:(mj + 1) * 512],
                        csb["dw2b8"][:, :, mj * 128:(mj + 1) * 128],
                        h1b8,
                        start=False, stop=True, perf_mode=DR)

                # logits: lg' = 256*lg = we18^T t28 + fw8^T h1b8
                pl = psM.tile([2 * G, RT], F32, tag="misc",
                              padded_shape=[128, 1024])
                nc.tensor.matmul(pl[:], csb["we18"][:], t28[:],
                                 start=True, stop=False, perf_mode=DR)
                nc.tensor.matmul(pl[:], csb["fw8"][:], h1b8,
                                 start=False, stop=True, perf_mode=DR)
                hw = spool.tile([G, RT], BF16, tag="hw")
                nc.scalar.activation(hw[:], pl[0:G, :], AF.Relu,
                                     bias=csb["twe"][:], scale=csb["swe"][:])
                pl2 = psM.tile([G, RT], F32, tag="misc",
                               padded_shape=[128, 1024])
                nc.tensor.matmul(pl2[:], csb["we2"][:], hw[:],
                                 start=True, stop=True)
                e = spool.tile([G, RT], BF16, tag="e")
                nc.scalar.activation(e[:], pl2[:], AF.Exp,
                                     bias=csb["web2"][:], scale=1.0)
                e8 = spool.tile([G, NPT, S // 2], F32, tag="e8")
                ev = e[:].rearrange("p (n s) -> p n s", s=S)
                nc.gpsimd.tensor_add(e8[:], ev[:, :, 0:8], ev[:, :, 8:16])
                e4 = spool.tile([G, NPT, S // 4], F32, tag="e4")
                nc.gpsimd.tensor_add(e4[:], e8[:, :, 0:4], e8[:, :, 4:8])
                nc.vector.reduce_sum(
                    esumacc[:, pt0:pt0 + NPT], e4[:], axis=AX.X)

                # expand e over channel groups; evacuate to SBUF on scalar
                pe = psM.tile([128, 1024], F32, tag="misc")
                for mj in range(2):
                    nc.tensor.matmul(
                        pe[:, mj * 512:(mj + 1) * 512],
                        csb["eoh"][:, mj, :], e[:],
                        start=True, stop=True)
                ebb = wpool.tile([128, 1024], BF16, tag="ebb")
                nc.vector.tensor_scalar_mul(ebb[:], pe[:], 1.0)

                # emit next tile's h1 stage (keeps PE dense, acts early)
                if t + 1 < NTILES:
                    h18_cur = emit_h1(t + 1)

                # previous tile's prod/outp (keeps DVE queue stall-free)
                if prev is not None:
                    emit_prod(*prev)
                prev = (pv, ebb, t)

            emit_prod(*prev)

            nc.sync.dma_start(out_d[:], outacc[:])
            nc.sync.dma_start(esum_d[:], esumacc[:])

    nc.compile()
    return nc


def host_prep(inputs, h1_fp8=True):
    """Fold BN, scale/cast weights to fp8, build per-core input maps."""
    f = {k: np.asarray(v) for k, v in inputs.items()}
    feats, coords, index = f["feats"], f["coords"], f["index"]
    index = index.astype(np.int64)

    s_q, t_q = _affine(f["bnq"], f["bq"])
    s_k, t_k = _affine(f["bnk"], f["bk"])
    s_hm, t_hm = _affine(f["pm_bn"], f["pm_b1"])
    s_hb, t_hb = _affine(f["pb_bn"], f["pb_b1"])

    b2b_we = np.asarray(f["pb_b2"], np.float32) @ np.asarray(f["we_w1"], np.float32)
    s_we, t_we = _affine(f["we_bn"], np.asarray(f["we_b1"], np.float32) + b2b_we)

    wk_eff = np.asarray(f["wk"], np.float32) * s_k[None, :]
    F_mat = np.asarray(f["pb_w2"], np.float32) @ np.asarray(f["we_w1"], np.float32)

    # w1' = w1 * bn_scale with ones-row bias fold (pos row 3 == 1)
    def w1_fold(w1, s_h, t_h):
        w = np.asarray(w1, np.float32) * s_h[None, :]
        return np.concatenate([w, t_h[None, :]], 0)  # [4, C]

    w1m_f = w1_fold(f["pm_w1"], s_hm, t_hm)
    w1b_f = w1_fold(f["pb_w1"], s_hb, t_hb)

    eoh = np.zeros((G, 2, 128), np.float32)
    for g in range(G):
        j, p0 = divmod(g * 32, 128)
        eoh[g, j, p0:p0 + 32] = 1.0

    shared = {
        "wq8": (_as_lhst(f["wq"]) * S_QW).astype(NP_FP8),
        "wke8": (_as_lhst(wk_eff) * S_KW).astype(NP_FP8),
        "wv8": (_as_lhst(f["wv"]) * S_VAL).astype(NP_FP8),
        "w2m8": (_as_lhst(f["pm_w2"]) * S_W2M).astype(NP_FP8),
        "w2b8": (_as_lhst(f["pb_w2"]) * S_VAL).astype(NP_FP8),
        "dw2b8": (_as_lhst(f["pb_w2"]) * S_VAL
                  - (_as_lhst(f["pb_w2"]) * S_VAL).astype(NP_FP8)
                  .astype(np.float32)).astype(NP_FP8),
        "we18": np.concatenate(
            [(_as_lhst(f["we_w1"]) * S_WE1), np.zeros((128, 2, G), np.float32)],
            axis=2).astype(NP_FP8),
        "fw8": np.concatenate(
            [(_as_lhst(F_mat) * S_FW), np.zeros((128, 2, G), np.float32)],
            axis=2).astype(NP_FP8),
        "sqv": _per_part(s_q * S_KW / S_QW),
        "tqv": _per_part(t_q * S_KW),
        "tkv": _per_part(t_k * S_KW),
        "b2v": _per_part(np.asarray(f["pm_b2"], np.float32) * S_W2M),
        "swe": (s_we / (S_WE1 * S_KW * S_W2M)).reshape(G, 1).astype(np.float32),
        "twe": t_we.reshape(G, 1).astype(np.float32),
        "we2": np.asarray(f["we_w2"], np.float32).astype(NP_BF16),
        "web2": np.asarray(f["we_b2"], np.float32).reshape(G, 1),
        "eoh": eoh.astype(NP_BF16),
    }
    if h1_fp8:
        # k = i*2 + p mapping for [2, 2, C] lhsT / [2, 2, cols] rhs
        def pack22(w4):  # [4, C] -> [2, 2, C]
            return np.ascontiguousarray(
                w4.reshape(2, 2, -1).transpose(1, 0, 2))
        shared["w1m8"] = (pack22(w1m_f) * S_W1).astype(NP_FP8)
        shared["w1b8"] = (pack22(w1b_f) * S_W1).astype(NP_FP8)
    else:
        shared["w1m"] = w1m_f.astype(NP_BF16)
        shared["w1b"] = w1b_f.astype(NP_BF16)

    in_maps = []
    for core in range(NCORES):
        b, qc = divmod(core, CPB)
        qoff = qc * NLOC
        fb32 = np.asarray(feats[b], np.float32)
        # featsT8: [128, 2, NLOC], [p, j, n] = feats[n, j*128+p]
        fq = fb32[qoff:qoff + NLOC]
        featsT = np.ascontiguousarray(
            fq.T.reshape(2, 128, NLOC).transpose(1, 0, 2)).astype(NP_FP8)
        idx = index[b, qoff:qoff + NLOC, :].reshape(-1)
        fg = fb32[idx]                                   # [NLOC*S, C]
        fgb8 = np.ascontiguousarray(
            fg.T.reshape(2, 128, NLOC * S).transpose(1, 0, 2)).astype(NP_FP8)
        cb = np.asarray(coords[b], np.float32)
        pos = cb[qoff:qoff + NLOC][:, None, :] - cb[idx.reshape(NLOC, S)]
        pos4 = np.concatenate(
            [pos.reshape(NLOC * S, 3).T,
             np.ones((1, NLOC * S), np.float32)], 0)     # [4, NLOC*S]
        m = dict(shared)
        m["featsT8"] = featsT
        m["fgb8"] = fgb8
        if h1_fp8:
            m["pos8"] = np.ascontiguousarray(
                pos4.reshape(2, 2, NLOC * S).transpose(1, 0, 2)).astype(NP_FP8)
        else:
            m["pos4"] = pos4.astype(NP_BF16)
        in_maps.append(m)

    # host-side unshard constants
    bias = (np.asarray(f["pb_b2"], np.float32)
            + np.asarray(f["bv"], np.float32))           # [C]
    return in_maps, bias


_NC_CACHE = {}


def _h1_fp8():
    return os.environ.get("KERNEL_H1BF16", "1") != "1"


def _get_program():
    key = "nc" + ("8" if _h1_fp8() else "16")
    if key not in _NC_CACHE:
        _NC_CACHE[key] = build_program(h1_fp8=_h1_fp8())
    return _NC_CACHE[key]


def unshard(results, bias):
    out = np.zeros((B, N, C), np.float32)
    for core in range(NCORES):
        b, qc = divmod(core, CPB)
        o = np.asarray(results[core]["out"], np.float32)    # [128, T, 2, n]
        es = np.asarray(results[core]["esum"], np.float32)  # [G, NLOC]
        # channel c = j*128 + p ; group g = c // 32 ; point = t*NPT + n
        oc = o.transpose(1, 3, 2, 0).reshape(NLOC, C)       # [n, c]
        denom = np.repeat(es.T, C // G, axis=1) * S_VAL     # [n, c]
        out[b, qc * NLOC:(qc + 1) * NLOC] = oc / denom + bias[None, :]
    return out


def kernel(**inputs):
    nc = _get_program()
    in_maps, bias = host_prep(inputs, h1_fp8=_h1_fp8())
    res = bass_utils.run_bass_kernel_spmd(
        nc, in_maps, list(range(NCORES)),
        trace=bool(int(os.environ.get("KERNEL_TRACE", "0"))))
    _NC_CACHE["last_results"] = res
    return unshard(res.results, bias)


# revision 26
# speedup vs baseline: 2.2151x; 1.0285x over previous
"""Trainium2 Bass kernel for GroupedVectorSA (gnn message passing), v2.

Self-contained: accepts FULL inputs (as produced by setup_inputs()), shards
across 8 NeuronCores (batch b = core//4, quarter of N = core%4), runs one
SPMD Bass/Tile program via bass_utils.run_bass_kernel_spmd, reassembles the
full [B, N, C] output.

v2 design notes (vs v1 baseline ~700us):
  - All K=256 projections run as fp8e4(e4m3) DoubleRow matmuls (0.5 cyc/row).
  - Host pre-transposes every operand (no on-device DMA transposes) and
    pre-gathers neighbor feature rows (input-only work).
  - Linear biases enter PSUM via K=1 ones-row matmuls; BN affines fold into
    weights host-side; per-matrix power-of-2 scales keep fp8 operands in
    range and are exactly compensated downstream.
  - relu(kg')-q fused into one DVE scalar_tensor_tensor from PSUM.
  - val = vg + peb accumulated inside PSUM by the PE (no vector add).
  - softmax: unnormalized e drives the weighted sum; the denominator
    (esum) ships to HBM and the host divides during the unshard, along
    with the constant output bias (pb_b2 + bv).
  - PSUM choreography: one rotating 3-buffer [128,1024] tag for all
    short-lived psum tensors + 1 buffer for val (lives across the softmax);
    prod/outp run software-pipelined one tile behind so the in-order DVE
    queue (t1, t2, prod) never stalls.
  - Engine balance: Scalar h1 acts + hw/exp; DVE t1/t2/prod; Pool esum +
    S-window output reduce.
"""

import os
import sys

import numpy as np

try:
    import concourse  # noqa: F401
except ImportError:
    sys.path.insert(0, "/opt/trn_rl_repo")

import ml_dtypes

import concourse.bacc as bacc
import concourse.bass as bass  # noqa: F401
import concourse.mybir as mybir
import concourse.tile as tile
from concourse import bass_utils

F32 = mybir.dt.float32
BF16 = mybir.dt.bfloat16
FP16 = mybir.dt.float16
FP8 = mybir.dt.float8e4

NP_BF16 = ml_dtypes.bfloat16
NP_FP8 = ml_dtypes.float8_e4m3fn

EPS = 1e-5
B, N, S, C, G = 2, 4096, 16, 256, 8
NCORES = 8
CPB = NCORES // B          # cores per batch = 4
NLOC = N // CPB            # points per core = 1024
NPT = 32                   # points per compute tile
RT = NPT * S               # gathered rows per compute tile = 512
NTILES = NLOC // NPT       # 32
NCHUNK = 4                 # fgT8 DMA chunks
TPC = NTILES // NCHUNK     # tiles per chunk = 8
CCOLS = NLOC * S // NCHUNK  # columns per chunk = 4096

# power-of-2 fp8 range scales (exactly compensated downstream)
S_QW = 8.0     # wq
S_KW = 4.0     # wk_eff; kg-psum, t1, qm carry S_KW
S_W2M = 1.0    # pm_w2 now bf16; t2 carries S_KW
S_WE1 = 1.0    # we_w1 bf16
S_FW = S_KW    # F bf16 must match t2's S_KW
S_VAL = 8.0    # wv and pb_w2 (val-psum, outacc carry S_VAL)
S_W1 = 4.0     # pm_w1/pb_w1 fp8 lhsT scale (h1 act divides back)

AO = mybir.AluOpType
AF = mybir.ActivationFunctionType
AX = mybir.AxisListType
DR = mybir.MatmulPerfMode.DoubleRow


def _affine(bn_p, lin_b):
    """Fold eval-mode BN (+ preceding linear bias) into scale/bias vectors."""
    bn_p = np.asarray(bn_p, np.float32)
    g, beta, m, v = bn_p[0], bn_p[1], bn_p[2], bn_p[3]
    s = g / np.sqrt(v + EPS)
    t = (np.asarray(lin_b, np.float32) - m) * s + beta
    return s.astype(np.float32), t.astype(np.float32)


def _as_lhst(w):
    """[256, X] -> [128, 2, X] (partition p, k-half j: k = j*128 + p)."""
    w = np.asarray(w, np.float32)
    return np.ascontiguousarray(w.reshape(2, 128, w.shape[1]).transpose(1, 0, 2))


def _per_part(vec):
    """[256] -> [128, 2]  (channel = j*128 + p)."""
    return np.ascontiguousarray(np.asarray(vec, np.float32).reshape(2, 128).T)


def build_program(h1_fp8=True):
    nc = bacc.Bacc("TRN2", target_bir_lowering=False, debug=False,
                   num_devices=NCORES)

    def din(name, shape, dt):
        return nc.dram_tensor(name, list(shape), dt, kind="ExternalInput")

    featsT8 = din("featsT8", [128, 2, NLOC], FP8)
    fgb8 = din("fgb8", [128, 2, NLOC * S], FP8)
    if h1_fp8:
        pos_d = din("pos8", [2, 2, NLOC * S], FP8)
    else:
        pos_d = din("pos4", [4, NLOC * S], BF16)
    consts = [
        ("wq8", [128, 2, C], FP8), ("wke8", [128, 2, C], FP8),
        ("wv8", [128, 2, C], FP8),
        ("w2m", [128, 2, C], BF16), ("w2b", [128, 2, C], BF16),
        ("we1", [128, 2, 2 * G], BF16), ("fw", [128, 2, 2 * G], BF16),
        ("sqv", [128, 2], F32), ("tqv", [128, 2], F32),
        ("tkv", [128, 2], F32), ("b2v", [128, 2], F32),
        ("swe", [G, 1], F32), ("twe", [G, 1], F32),
        ("we2", [G, G], BF16), ("web2", [G, 1], F32),
        ("eoh", [G, 2, 128], BF16),
    ]
    if h1_fp8:
        consts += [("w1m8", [2, 2, C], FP8), ("w1b8", [2, 2, C], FP8)]
    else:
        consts += [("w1m", [4, C], BF16), ("w1b", [4, C], BF16)]
    cdram = {name: din(name, shape, dt) for name, shape, dt in consts}

    out_d = nc.dram_tensor("out", [128, NTILES, 2, NPT], F32,
                           kind="ExternalOutput")
    esum_d = nc.dram_tensor("esum", [G, NLOC], F32, kind="ExternalOutput")

    with tile.TileContext(nc) as tc:
        with (
            tc.tile_pool(name="const", bufs=1) as cpool,
            tc.tile_pool(name="big", bufs=1) as bigpool,
            tc.tile_pool(name="work", bufs=2) as wpool,
            tc.tile_pool(name="small", bufs=3) as spool,
            tc.tile_pool(name="ps", bufs=2, space="PSUM") as psP,
            tc.tile_pool(name="psm", bufs=1, space="PSUM") as psM,
        ):
            csb = {}
            for name, shape, dt in consts:
                t = cpool.tile(list(shape), dt, tag=name)
                nc.sync.dma_start(t[:], cdram[name][:])
                csb[name] = t

            featsT = cpool.tile([128, 2, NLOC], FP8, tag="featsT")
            nc.sync.dma_start(featsT[:], featsT8[:])
            posT = cpool.tile(
                [2, 2, NLOC * S] if h1_fp8 else [4, NLOC * S],
                FP8 if h1_fp8 else BF16, tag="posT")
            nc.sync.dma_start(posT[:], pos_d[:])
            fgc = []
            for cch in range(NCHUNK):
                t = cpool.tile([128, 2, CCOLS], FP8, tag=f"fg{cch}")
                nc.sync.dma_start(t[:], fgb8[:, :, cch * CCOLS:(cch + 1) * CCOLS])
                fgc.append(t)

            # tile-local layout [p, tile, j, n] so (j, n) flattens contiguous
            outacc = bigpool.tile([128, NTILES, 2, NPT], F32, tag="outacc")
            esumacc = bigpool.tile([G, NLOC], F32, tag="esumacc")

            # ---- q phase: qm = S_KW * relu(bn_q(feats @ wq + bq)) ----------
            # tile-local layout [p, tile, j, n] so (j, n) flattens contiguous
            qm = bigpool.tile([128, NTILES, 2, NPT], BF16, tag="qm")
            tpch = 512 // NPT  # tiles per 512-point chunk
            for ch in range(NLOC // 512):
                pq = psP.tile([128, 1024], F32, tag="rot")
                for mj in range(2):
                    nc.tensor.matmul(
                        pq[:, mj * 512:(mj + 1) * 512],
                        csb["wq8"][:, :, mj * 128:(mj + 1) * 128],
                        featsT[:, :, ch * 512:(ch + 1) * 512],
                        start=True, stop=True, perf_mode=DR)
                for mj in range(2):
                    nc.scalar.activation(
                        qm[:, ch * tpch:(ch + 1) * tpch, mj, :],
                        pq[:, mj * 512:(mj + 1) * 512]
                            .rearrange("p (t n) -> p t n", n=NPT),
                        AF.Relu,
                        bias=csb["tqv"][:, mj:mj + 1],
                        scale=csb["sqv"][:, mj:mj + 1])

            # ---- main tile loop ------------------------------------------
            # h1 stage runs one tile AHEAD; prod/outp one tile BEHIND.
            prev = None  # (pv, ebb, ti) of previous tile

            def emit_h1(t):
                g0 = t * RT
                h18 = wpool.tile([128, 2, 2, 512], BF16, tag="h18")
                for mj in range(2):
                    ph = psP.tile([128, 1024], F32, tag="rot")
                    for mlp, wkey in ((0, "m"), (1, "b")):
                        if h1_fp8:
                            nc.tensor.matmul(
                                ph[:, mlp * 512:(mlp + 1) * 512],
                                csb[f"w1{wkey}8"][:, :, mj * 128:(mj + 1) * 128],
                                posT[:, :, g0:g0 + RT],
                                start=True, stop=True, perf_mode=DR)
                        else:
                            nc.tensor.matmul(
                                ph[:, mlp * 512:(mlp + 1) * 512],
                                csb[f"w1{wkey}"][:, mj * 128:(mj + 1) * 128],
                                posT[:, g0:g0 + RT],
                                start=True, stop=True)
                    nc.scalar.activation(
                        h18[:, :, mj, :], ph[:].rearrange("p (l n) -> p l n", l=2),
                        AF.Relu, scale=1.0 / S_W1 if h1_fp8 else 1.0)
                return h18

            def emit_prod(valb, ebb, ti):
                # prod = val'(SBUF) * ebb(SBUF)  on DVE
                prod = wpool.tile([128, 64, S], BF16, tag="prod")
                nc.vector.scalar_tensor_tensor(
                    prod[:], valb[:].rearrange("p (m s) -> p m s", s=S),
                    0.0, ebb[:].rearrange("p (m s) -> p m s", s=S),
                    op0=AO.add, op1=AO.mult)
                # S-window reduce: pool halving adds, then tiny DVE reduce
                p8 = wpool.tile([128, 64, S // 2], F32, tag="p8")
                nc.gpsimd.tensor_add(p8[:], prod[:, :, 0:8], prod[:, :, 8:16])
                p4 = wpool.tile([128, 64, S // 4], F32, tag="p4")
                nc.gpsimd.tensor_add(p4[:], p8[:, :, 0:4], p8[:, :, 4:8])
                p2 = wpool.tile([128, 64, S // 8], F32, tag="p2")
                nc.gpsimd.tensor_add(p2[:], p4[:, :, 0:2], p4[:, :, 2:4])
                nc.vector.reduce_sum(
                    outacc[:, ti, :, :].rearrange("p j n -> p (j n)"),
                    p2[:], axis=AX.X)

            h18_cur = emit_h1(0)

            for t in range(NTILES):
                pt0 = t * NPT
                fg = fgc[t // TPC]
                r0 = (t % TPC) * RT

                # pos-path hidden layers were computed one tile ahead
                h18 = h18_cur
                h1m8 = h18[:, 0, :, :]
                h1b8 = h18[:, 1, :, :]

                # kg' = S_KW*(wk_eff @ fg + tk); t1 = relu(kg') - qm
                pk = psP.tile([128, 1024], F32, tag="rot")
                for mj in range(2):
                    nc.tensor.matmul(
                        pk[:, mj * 512:(mj + 1) * 512],
                        csb["wke8"][:, :, mj * 128:(mj + 1) * 128],
                        fg[:, :, r0:r0 + RT],
                        start=True, stop=True, perf_mode=DR)
                kgt = wpool.tile([128, 2, RT], BF16, tag="kgt")
                for mj in range(2):
                    nc.scalar.activation(
                        kgt[:, mj, :], pk[:, mj * 512:(mj + 1) * 512],
                        AF.Relu, bias=csb["tkv"][:, mj:mj + 1])
                t1 = wpool.tile([128, 64, S], BF16, tag="t1")
                qb = qm[:, t, :, :].rearrange("p j n -> p (j n)") \
                    .unsqueeze(2).broadcast_to((128, 64, S))
                nc.vector.scalar_tensor_tensor(
                    t1[:], kgt[:].rearrange("p j (n s) -> p (j n) s", s=S),
                    0.0, qb, op0=AO.add, op1=AO.subtract)

                # pem' = S_W2M*(pem + b2m); t2 = pem' * t1 (carries 32x)
                pp = psP.tile([128, 1024], F32, tag="rot")
                for mj in range(2):
                    for kt in range(2):
                        nc.tensor.matmul(
                            pp[:, mj * 512:(mj + 1) * 512],
                            csb["w2m"][:, kt, mj * 128:(mj + 1) * 128],
                            h1m8[:, kt, :],
                            start=(kt == 0), stop=(kt == 1))
                pemb = wpool.tile([128, 2, RT], BF16, tag="pemb")
                for mj in range(2):
                    nc.scalar.activation(
                        pemb[:, mj, :], pp[:, mj * 512:(mj + 1) * 512],
                        AF.Identity, bias=csb["b2v"][:, mj:mj + 1])
                t28 = wpool.tile([128, 2, 512], BF16, tag="t28")
                nc.vector.scalar_tensor_tensor(
                    t28[:], pemb[:],
                    0.0, t1[:].rearrange("p m s -> p (m s)")
                        .rearrange("p (j n) -> p j n", j=2),
                    op0=AO.add, op1=AO.mult)

                # val' = S_VAL*(vg + peb0): both accumulated in PSUM
                pv = psP.tile([128, 1024], F32, tag="pv", bufs=1)
                for mj in range(2):
                    nc.tensor.matmul(
                        pv[:, mj * 512:(mj + 1) * 512],
                        csb["wv8"][:, :, mj * 128:(mj + 1) * 128],
                        fg[:, :, r0:r0 + RT],
                        start=True, stop=False, perf_mode=DR)
                    for kt in range(2):
                        nc.tensor.matmul(
                            pv[:, mj * 512:(mj + 1) * 512],
                            csb["w2b"][:, kt, mj * 128:(mj + 1) * 128],
                            h1b8[:, kt, :],
                            start=False, stop=(kt == 1))
                valb = wpool.tile([128, 1024], BF16, tag="valb")
                nc.vector.tensor_scalar_mul(valb[:], pv[:], 1.0)

                # logits: lg' = 256*lg = we18^T t28 + fw8^T h1b8
                pl = psM.tile([2 * G, RT], F32, tag="misc",
                              padded_shape=[128, 1024])
                for kt in range(2):
                    nc.tensor.matmul(pl[:], csb["we1"][:, kt, :], t28[:, kt, :],
                                     start=(kt == 0), stop=False)
                for kt in range(2):
                    nc.tensor.matmul(pl[:], csb["fw"][:, kt, :], h1b8[:, kt, :],
                                     start=False, stop=(kt == 1))
                hw = spool.tile([G, RT], BF16, tag="hw")
                nc.scalar.activation(hw[:], pl[0:G, :], AF.Relu,
                                     bias=csb["twe"][:], scale=csb["swe"][:])
                pl2 = psM.tile([G, RT], F32, tag="misc",
                               padded_shape=[128, 1024])
                nc.tensor.matmul(pl2[:], csb["we2"][:], hw[:],
                                 start=True, stop=True)
                e = spool.tile([G, RT], BF16, tag="e")
                nc.scalar.activation(e[:], pl2[:], AF.Exp,
                                     bias=csb["web2"][:], scale=1.0)
                e8 = spool.tile([G, NPT, S // 2], F32, tag="e8")
                ev = e[:].rearrange("p (n s) -> p n s", s=S)
                nc.gpsimd.tensor_add(e8[:], ev[:, :, 0:8], ev[:, :, 8:16])
                e4 = spool.tile([G, NPT, S // 4], F32, tag="e4")
                nc.gpsimd.tensor_add(e4[:], e8[:, :, 0:4], e8[:, :, 4:8])
                nc.vector.reduce_sum(
                    esumacc[:, pt0:pt0 + NPT], e4[:], axis=AX.X)

                # expand e over channel groups; evacuate to SBUF on scalar
                pe = psM.tile([128, 1024], F32, tag="misc")
                for mj in range(2):
                    nc.tensor.matmul(
                        pe[:, mj * 512:(mj + 1) * 512],
                        csb["eoh"][:, mj, :], e[:],
                        start=True, stop=True)
                ebb = wpool.tile([128, 1024], BF16, tag="ebb")
                nc.vector.tensor_scalar_mul(ebb[:], pe[:], 1.0)

                # emit next tile's h1 stage (keeps PE dense, acts early)
                if t + 1 < NTILES:
                    h18_cur = emit_h1(t + 1)

                # previous tile's prod/outp (keeps DVE queue stall-free)
                if prev is not None:
                    emit_prod(*prev)
                prev = (valb, ebb, t)

            emit_prod(*prev)

            nc.sync.dma_start(out_d[:], outacc[:])
            nc.sync.dma_start(esum_d[:], esumacc[:])

    nc.compile()
    return nc


def host_prep(inputs, h1_fp8=True):
    """Fold BN, scale/cast weights to fp8, build per-core input maps."""
    f = {k: np.asarray(v) for k, v in inputs.items()}
    feats, coords, index = f["feats"], f["coords"], f["index"]
    index = index.astype(np.int64)

    s_q, t_q = _affine(f["bnq"], f["bq"])
    s_k, t_k = _affine(f["bnk"], f["bk"])
    s_hm, t_hm = _affine(f["pm_bn"], f["pm_b1"])
    s_hb, t_hb = _affine(f["pb_bn"], f["pb_b1"])

    b2b_we = np.asarray(f["pb_b2"], np.float32) @ np.asarray(f["we_w1"], np.float32)
    s_we, t_we = _affine(f["we_bn"], np.asarray(f["we_b1"], np.float32) + b2b_we)

    wk_eff = np.asarray(f["wk"], np.float32) * s_k[None, :]
    F_mat = np.asarray(f["pb_w2"], np.float32) @ np.asarray(f["we_w1"], np.float32)

    # w1' = w1 * bn_scale with ones-row bias fold (pos row 3 == 1)
    def w1_fold(w1, s_h, t_h):
        w = np.asarray(w1, np.float32) * s_h[None, :]
        return np.concatenate([w, t_h[None, :]], 0)  # [4, C]

    w1m_f = w1_fold(f["pm_w1"], s_hm, t_hm)
    w1b_f = w1_fold(f["pb_w1"], s_hb, t_hb)

    eoh = np.zeros((G, 2, 128), np.float32)
    for g in range(G):
        j, p0 = divmod(g * 32, 128)
        eoh[g, j, p0:p0 + 32] = 1.0

    shared = {
        "wq8": (_as_lhst(f["wq"]) * S_QW).astype(NP_FP8),
        "wke8": (_as_lhst(wk_eff) * S_KW).astype(NP_FP8),
        "wv8": (_as_lhst(f["wv"]) * S_VAL).astype(NP_FP8),
        "w2m": (_as_lhst(f["pm_w2"]) * S_W2M).astype(NP_BF16),
        "w2b": (_as_lhst(f["pb_w2"]) * S_VAL).astype(NP_BF16),
        "we1": np.concatenate(
            [(_as_lhst(f["we_w1"]) * S_WE1), np.zeros((128, 2, G), np.float32)],
            axis=2).astype(NP_BF16),
        "fw": np.concatenate(
            [(_as_lhst(F_mat) * S_FW), np.zeros((128, 2, G), np.float32)],
            axis=2).astype(NP_BF16),
        "sqv": _per_part(s_q * S_KW / S_QW),
        "tqv": _per_part(t_q * S_KW),
        "tkv": _per_part(t_k * S_KW),
        "b2v": _per_part(np.asarray(f["pm_b2"], np.float32) * S_W2M),
        "swe": (s_we / (S_WE1 * S_KW * S_W2M)).reshape(G, 1).astype(np.float32),
        "twe": t_we.reshape(G, 1).astype(np.float32),
        "we2": np.asarray(f["we_w2"], np.float32).astype(NP_BF16),
        "web2": np.asarray(f["we_b2"], np.float32).reshape(G, 1),
        "eoh": eoh.astype(NP_BF16),
    }
    if h1_fp8:
        # k = i*2 + p mapping for [2, 2, C] lhsT / [2, 2, cols] rhs
        def pack22(w4):  # [4, C] -> [2, 2, C]
            return np.ascontiguousarray(
                w4.reshape(2, 2, -1).transpose(1, 0, 2))
        shared["w1m8"] = (pack22(w1m_f) * S_W1).astype(NP_FP8)
        shared["w1b8"] = (pack22(w1b_f) * S_W1).astype(NP_FP8)
    else:
        shared["w1m"] = w1m_f.astype(NP_BF16)
        shared["w1b"] = w1b_f.astype(NP_BF16)

    in_maps = []
    for core in range(NCORES):
        b, qc = divmod(core, CPB)
        qoff = qc * NLOC
        fb32 = np.asarray(feats[b], np.float32)
        # featsT8: [128, 2, NLOC], [p, j, n] = feats[n, j*128+p]
        fq = fb32[qoff:qoff + NLOC]
        featsT = np.ascontiguousarray(
            fq.T.reshape(2, 128, NLOC).transpose(1, 0, 2)).astype(NP_FP8)
        idx = index[b, qoff:qoff + NLOC, :].reshape(-1)
        fg = fb32[idx]                                   # [NLOC*S, C]
        fgb8 = np.ascontiguousarray(
            fg.T.reshape(2, 128, NLOC * S).transpose(1, 0, 2)).astype(NP_FP8)
        cb = np.asarray(coords[b], np.float32)
        pos = cb[qoff:qoff + NLOC][:, None, :] - cb[idx.reshape(NLOC, S)]
        pos4 = np.concatenate(
            [pos.reshape(NLOC * S, 3).T,
             np.ones((1, NLOC * S), np.float32)], 0)     # [4, NLOC*S]
        m = dict(shared)
        m["featsT8"] = featsT
        m["fgb8"] = fgb8
        if h1_fp8:
            m["pos8"] = np.ascontiguousarray(
                pos4.reshape(2, 2, NLOC * S).transpose(1, 0, 2)).astype(NP_FP8)
        else:
            m["pos4"] = pos4.astype(NP_BF16)
        in_maps.append(m)

    # host-side unshard constants
    bias = (np.asarray(f["pb_b2"], np.float32)
            + np.asarray(f["bv"], np.float32))           # [C]
    return in_maps, bias


_NC_CACHE = {}


def _h1_fp8():
    return os.environ.get("KERNEL_H1BF16", "1") != "1"


def _get_program():
    key = "nc" + ("8" if _h1_fp8() else "16")
    if key not in _NC_CACHE:
        _NC_CACHE[key] = build_program(h1_fp8=_h1_fp8())
    return _NC_CACHE[key]


def unshard(results, bias):
    out = np.zeros((B, N, C), np.float32)
    for core in range(NCORES):
        b, qc = divmod(core, CPB)
        o = np.asarray(results[core]["out"], np.float32)    # [128, T, 2, n]
        es = np.asarray(results[core]["esum"], np.float32)  # [G, NLOC]
        # channel c = j*128 + p ; group g = c // 32 ; point = t*NPT + n
        oc = o.transpose(1, 3, 2, 0).reshape(NLOC, C)       # [n, c]
        denom = np.repeat(es.T, C // G, axis=1) * S_VAL     # [n, c]
        out[b, qc * NLOC:(qc + 1) * NLOC] = oc / denom + bias[None, :]
    return out


def kernel(**inputs):
    nc = _get_program()
    in_maps, bias = host_prep(inputs, h1_fp8=_h1_fp8())
    res = bass_utils.run_bass_kernel_spmd(
        nc, in_maps, list(range(NCORES)),
        trace=bool(int(os.environ.get("KERNEL_TRACE", "0"))))
    _NC_CACHE["last_results"] = res
    return unshard(res.results, bias)
